# revision 23
# baseline (speedup 1.0000x reference)
"""Trainium2 Bass kernel: single-head causal attention, data-parallel over batch.

Per core (one batch element):
    Q = x @ w_q; K = x @ w_k; V = (x @ w_v1) @ w_v2
    out = softmax_causal(Q K^T / sqrt(64)) @ V

Sharding: batch 8 -> one element per NeuronCore, weights replicated.

Design notes (v2 of this kernel; hardware-measured 76.6us vs 84.5us v1):
- Low-rank reassociation: attn @ V = (attn @ Vp) @ w_v2 (rank 64), so the
  numerator GEMM contracts to width 64 instead of 1024.
- Scores computed transposed (S^T = K Q^T) so P^T = exp(S^T) lands in the
  lhsT layout of the numerator matmul; a ones column on Vp makes row 64
  of the numerator the softmax denominator for free.
- PE-array ROW TILING for the K=64 matmuls: score strips are emitted as
  concurrent pairs - even strip in array rows 0-63 (kt_sb + Q^T at
  partitions 0-63), odd strip in rows 64-127 (K^T read in place from
  qkt_sb[64:128] + a re-based Q^T copy at partitions 64-127). Verified on
  HW: the pair's matmuls start ~6ns apart when no semaphore wait blocks
  the second. The pair lands in one [128,1024] PSUM tile spanning two
  banks, so a single ACT exp drains both strips (ACT instruction count
  nearly halves - ACT is the mid-phase critical engine).
- Out-GEMM (K=64) row-paired the same way: the two E-halves of a q-tile
  run in rows 0-63 / 64-127 against partition-replicated num / w_v2 into
  one [128,1024] pair, drained by ONE tensor_scalar with recip fused.
- PSUM: one double-buffered 2-bank pair pool shared by score and out
  pairs. bufs=1 self-chained producer->consumer->producer and left the
  PE idle long enough for the HAM clock gate to re-throttle to 1.2GHz
  (the dominant failure mode of every slower variant of this kernel).
- kt/qt_hi re-base copies ride the HWDGE queues (the SWDGE hop measured
  ~4us latency and stalled the first strip pairs of each group).
- Head: groups 2-3 deferred behind a WAW byte-dep on qkt so the early
  HBM window (8 cores contending, per-queue ~130GB/s) carries only
  groups 0-1; 36 N=256 warmup matmuls cover DMA-wait so the clock gate
  is at 8/8 when real work starts. N=64 fillers do NOT register as HAM
  activity - only chunky N>=256 matmuls hold the gate.
- Tail: the final epilogue's d4 round trip rides the sync HWDGE queue
  and is bridged by data-dependent cover matmuls; drains split ACT+DVE.
- Output written bf16 (host upcasts); well inside tolerance.
"""

import os
import sys

import numpy as np

for _p in ("/opt/trn_rl_repo", "/root/.axon_site/_ro/trn_rl_repo"):
    if os.path.isdir(_p) and _p not in sys.path:
        sys.path.insert(0, _p)
os.environ.setdefault("MYCRO_LOCAL_CACHE", "1")

import ml_dtypes  # noqa: E402
import concourse.bass as bass  # noqa: E402
import concourse.mybir as mybir  # noqa: E402
import concourse.tile as tile  # noqa: E402
from concourse import bacc  # noqa: E402
from concourse import bass_utils  # noqa: E402
from concourse.masks import make_identity, make_upper_triangular  # noqa: E402

F32 = mybir.dt.float32
BF16 = mybir.dt.bfloat16

B, S, E, D = 8, 2048, 1024, 64
P = 128
NS = S // P       # 16 s/q tiles
NE = E // P       # 8 E-chunks (projection contraction)
QG = 512          # q-group width
NQG = S // QG     # 4 q-groups
GT = QG // P      # 4 q-tiles per group
SCALE = D ** -0.5
EXP_FN = mybir.ActivationFunctionType.Exp
COPY_FN = mybir.ActivationFunctionType.Copy
NWARM = 36        # dummy matmuls to warm the PE clock gate during loads


def build_kernel(nc):
    # x pre-tiled on host: x_t[p, g*NE*QG + c*QG + s] = x[g*QG+s, c*128+p]
    x_t = nc.dram_tensor("x_t", (P, NQG * NE * QG), BF16,
                         kind="ExternalInput").ap()
    # w_qk pre-tiled: w_qk[p, c*128 + m] = [w_q*scale | w_k][c*128+p, m]
    w_qk = nc.dram_tensor("w_qk", (P, NE * P), BF16, kind="ExternalInput").ap()
    w_v1 = nc.dram_tensor("w_v1", (P, NE * D), BF16, kind="ExternalInput").ap()
    w_v2 = nc.dram_tensor("w_v2", (D, E), BF16, kind="ExternalInput").ap()
    out = nc.dram_tensor("out", (S, E), BF16, kind="ExternalOutput").ap()

    with tile.TileContext(nc) as tc:
        _body(tc, nc, x_t, w_qk, w_v1, w_v2, out)


def _body(tc, nc, x_t, w_qk, w_v1, w_v2, out):
    from contextlib import ExitStack

    with ExitStack() as ctx:
        const = ctx.enter_context(tc.tile_pool(name="const", bufs=1))
        big = ctx.enter_context(tc.tile_pool(name="big", bufs=1))
        # pt pairs live from exp until their last numerator read; sized
        # above peak-live so pool-reuse WARs never stall the producers
        ptp = ctx.enter_context(tc.tile_pool(name="ptp", bufs=14))
        outp = ctx.enter_context(tc.tile_pool(name="outp", bufs=6))
        small = ctx.enter_context(tc.tile_pool(name="small", bufs=8))
        # PSUM budget (8 banks): one double-buffered 2-bank pair pool
        # shared by score pairs AND out pairs (4) + psP 2x1 + psN 2x1.
        # Two slots mean a pair's consumer (exp / drain) never gates the
        # NEXT pair's matmuls - the engines each keep a backlog.
        psB = ctx.enter_context(tc.tile_pool(name="psB", bufs=2, space="PSUM"))
        psP = ctx.enter_context(tc.tile_pool(name="psP", bufs=2, space="PSUM"))
        psN = ctx.enter_context(tc.tile_pool(name="psN", bufs=2, space="PSUM"))

        # ---- warm-up operand: memset immediately, no DMA dependency ----
        wu = const.tile([P, 256], BF16, tag="wu")
        nc.vector.memset(wu[:, :], 0.001)

        # ---- weight + x^T loads ----
        xT = big.tile([P, NQG, NE, QG], BF16, tag="xT")
        xtv = x_t.rearrange("p (g c s) -> p g c s", g=NQG, c=NE)
        wqk_sb = const.tile([P, NE, P], BF16, tag="wqk")
        wv1_sb = const.tile([P, NE, D], BF16, tag="wv1")
        # w_v2 replicated into both partition halves: row-paired out GEMMs
        # read rhs at partitions 0-63 (eh=0) and 64-127 (eh=1)
        wv2_sb = const.tile([P, E], BF16, tag="wv2")
        # wqk split across both HWDGE queues so x group 0 doesn't wait
        # behind the full weight transfer
        wqkv = w_qk.rearrange("p (c m) -> p c m", m=P)
        h = NE // 2
        nc.sync.dma_start(wqk_sb[:, 0:h, :], wqkv[:, 0:h, :])
        nc.scalar.dma_start(wqk_sb[:, h:NE, :], wqkv[:, h:NE, :])
        # x group 0 as one half per HWDGE queue (4KB/partition lines)
        nc.scalar.dma_start(xT[:, 0, 0:h, :], xtv[:, 0, 0:h, :])
        nc.sync.dma_start(xT[:, 0, h:NE, :], xtv[:, 0, h:NE, :])
        hw_engs = (nc.sync, nc.scalar)
        # group 1 next on both queues - needed early (proj(1) in period 0)
        nc.scalar.dma_start(xT[:, 1, 0:h, :], xtv[:, 1, 0:h, :])
        nc.sync.dma_start(xT[:, 1, h:NE, :], xtv[:, 1, h:NE, :])
        nc.gpsimd.dma_start(wv1_sb[:, :, :],
                            w_v1.rearrange("p (c d) -> p c d", d=D))
        nc.gpsimd.dma_start(wv2_sb[0:D, :], w_v2)
        nc.gpsimd.dma_start(wv2_sb[D:P, :], w_v2)

        ident = const.tile([D, D], BF16, tag="ident")
        ident4 = const.tile([GT, GT], F32, tag="ident4")
        tri = const.tile([P, P], BF16, tag="tri")
        ones1 = const.tile([1, D], F32, tag="ones1")
        nc.vector.memset(ones1[:, :], 1.0)
        # tri[s, q] = 1 where s <= q else 0 (valid causal region, S^T layout)

        def emit_warm(n, width=256):
            """Dummy matmuls to keep the PE clock gate at 8/8."""
            psw = None
            for _ in range(n):
                psw = psP.tile([P, 256], F32, tag="psP")
                nc.tensor.matmul(psw[:, 0:width], wu[:, 0:P],
                                 wu[:, 0:width], start=True, stop=True)
            # token reader so the verifier sees the results consumed
            nc.vector.tensor_copy(wu[0:1, 0:1], psw[0:1, 0:1])

        # ---- PE warm-up: dummy matmuls while DMAs stream in ----
        # N=64 fillers were tried and do NOT register as busy in the HAM
        # activity window (50% duty cycle reads as idle); only chunky
        # N=256 back-to-back matmuls hold the clock gate.
        emit_warm(NWARM)

        qkt_sb = big.tile([P, S], BF16, tag="qkt")
        kt_sb = big.tile([D, S], BF16, tag="kt")
        # Q^T re-based to partitions 64-127 (rhs of odd score strips)
        qt_hi = big.tile([P, S], BF16, tag="qthi")
        vpt_sb = big.tile([D, S], BF16, tag="vpt")
        # Vp tile-wise as [s, 64+1] (numerator lhsT); ones column -> denom row
        vp_sb = big.tile([P, NS, D + 1], BF16, tag="vp")
        nc.vector.memset(vp_sb[:, :, D], 1.0)

        proj_ps = {}

        def proj_pass_qk(ng, part=None):
            """part=0/1 emits half the chunks (finer PE interleave
            granularity); part=None emits the whole pass."""
            sl = slice(ng * QG, (ng + 1) * QG)
            if part != 1:
                proj_ps["qk", ng] = psP.tile([P, QG], F32, tag="psP", name="psqk")
            ps = proj_ps["qk", ng]
            lo = 0 if part != 1 else NE // 2
            hi = NE // 2 if part == 0 else NE
            for ec in range(lo, hi):
                nc.tensor.matmul(
                    ps[:, :], wqk_sb[:, ec, :], xT[:, ng, ec, :],
                    start=(ec == 0), stop=(ec == NE - 1))
            if part == 0:
                return
            nc.vector.tensor_copy(qkt_sb[:, sl], ps[:, :])
            # strip pair operands: K^T at partitions 0-63 (even strips),
            # Q^T at partitions 64-127 (odd strips). HWDGE queues: the
            # SWDGE hop measured ~4us latency and stalled the first strips
            nc.sync.dma_start(kt_sb[:, sl], qkt_sb[D:P, sl])
            nc.scalar.dma_start(qt_hi[D:P, sl], qkt_sb[0:D, sl])

        def proj_pass_v1(ng, part=None):
            sl = slice(ng * QG, (ng + 1) * QG)
            if part != 1:
                proj_ps["v1", ng] = psP.tile([P, QG], F32, tag="psP", name="psv1")
            ps = proj_ps["v1", ng]
            lo = 0 if part != 1 else NE // 2
            hi = NE // 2 if part == 0 else NE
            for ec in range(lo, hi):
                nc.tensor.matmul(
                    ps[0:D, :], wv1_sb[:, ec, :], xT[:, ng, ec, :],
                    start=(ec == 0), stop=(ec == NE - 1))
            if part == 0:
                return
            nc.scalar.copy(vpt_sb[:, sl], ps[0:D, :])

        def vp_transp(ng, part=None):
            lo = ng * GT + (2 if part == 1 else 0)
            hi = ng * GT + (2 if part == 0 else GT)
            for st in range(lo, hi):
                pst = psP.tile([P, D], BF16, tag="psP")
                nc.tensor.transpose(pst[0:P, 0:D],
                                    vpt_sb[:, st * P:(st + 1) * P],
                                    ident[:, :])
                nc.vector.tensor_copy(vp_sb[:, st, 0:D], pst[0:P, 0:D])

        def _lo(qg, j):
            dt_blk = j - qg * GT
            return dt_blk * P if 0 < dt_blk < GT else 0

        def emit_strip_pair(qg, j):
            """Strips j (even) and j+1 as a concurrent row-tiled pair in
            one 2-bank PSUM tile, drained by a single fused exp when the
            written region is contiguous."""
            lo0, lo1 = _lo(qg, j), _lo(qg, j + 1)
            ps = psB.tile([P, 2 * QG], F32, tag="pair")
            # even strip -> array rows 0-63
            nc.tensor.matmul(
                ps[:, lo0:QG],
                kt_sb[:, j * P:(j + 1) * P],
                qkt_sb[0:D, qg * QG + lo0:(qg + 1) * QG],
                start=True, stop=True,
            )
            # odd strip -> array rows 64-127 (K^T in place, Q^T replica)
            nc.tensor.matmul(
                ps[:, QG + lo1:2 * QG],
                qkt_sb[D:P, (j + 1) * P:(j + 2) * P],
                qt_hi[D:P, qg * QG + lo1:(qg + 1) * QG],
                start=True, stop=True,
            )
            pt = ptp.tile([P, 2 * QG], BF16, tag="pt")
            if lo0 == 0 and lo1 == 0:
                nc.scalar.activation(pt[:, :], ps[:, :], EXP_FN)
            else:
                nc.scalar.activation(pt[:, lo0:QG], ps[:, lo0:QG], EXP_FN)
                nc.scalar.activation(pt[:, QG + lo1:], ps[:, QG + lo1:],
                                     EXP_FN)
            out = []
            for jj, off, lo in ((j, 0, lo0), (j + 1, QG, lo1)):
                dt_blk = jj - qg * GT
                if 0 <= dt_blk < GT:
                    # mask the diagonal 128x128 block (cols < lo are never
                    # read: numerator MMs are lo-trimmed)
                    nc.gpsimd.tensor_mul(
                        pt[:, off + dt_blk * P:off + (dt_blk + 1) * P],
                        pt[:, off + dt_blk * P:off + (dt_blk + 1) * P],
                        tri[:, :],
                    )
                out.append((jj, pt[:, off + lo:off + QG], lo))
            return out

        def emit_epilogue(qg, psn, cover=False):
            """Denominator row -> per-partition recip; numerator -> bf16
            replicated to both partition halves (row-paired out GEMM).
            cover=True: bridge the d4 DMA round trip with dummy matmuls
            that READ d_sb so the scheduler cannot hoist them."""
            d_sb = small.tile([1, QG], F32, tag="dsb")
            if cover:  # tail: ACT is free; DVE still drains out tiles
                nc.scalar.copy(d_sb[:, :], psn[D:D + 1, :])
            else:
                nc.vector.tensor_copy(d_sb[:, :], psn[D:D + 1, :])
            d4 = small.tile([GT, P], F32, tag="d4")
            # sync HWDGE always: the SWDGE hop's ~4us latency delivered
            # recip a period late and stalled the out pairs behind it
            nc.sync.dma_start(d4[:, :], d_sb[0:1, :])
            if cover:
                psw = None
                for _ in range(5):
                    psw = psP.tile([P, 256], F32, tag="psP")
                    nc.tensor.matmul(psw[0:D, :], ones1[:, :],
                                     d_sb[0:1, 0:256],
                                     start=True, stop=True)
                nc.vector.tensor_copy(d_sb[0:1, 0:1], psw[0:1, 0:1])
            ps4 = psP.tile([P, GT], F32, tag="psP")
            nc.tensor.transpose(ps4[0:P, 0:GT], d4[:, :], ident4[:, :])
            recip = small.tile([P, GT], F32, tag="recip")
            nc.vector.reciprocal(recip[:, :], ps4[0:P, 0:GT])
            num_sb = small.tile([P, QG], BF16, tag="numsb")
            if cover:
                # tail critical path: produce both halves immediately on
                # the two drain engines in parallel
                nc.vector.tensor_copy(num_sb[0:D, :], psn[0:D, :])
                nc.scalar.copy(num_sb[D:P, :], psn[0:D, :])
            else:
                nc.vector.tensor_copy(num_sb[0:D, :], psn[0:D, :])
                # replica to partitions 64-127 rides the SWDGE queue
                nc.gpsimd.dma_start(num_sb[D:P, :], num_sb[0:D, :])
            return num_sb, recip

        def out_pair(qg, t, num_sb, recip, drain="v", split=False):
            """One q-tile's two E-halves as a concurrent row-tiled pair,
            drained by a single fused tensor_scalar (recip folded in)."""
            i = qg * GT + t  # global q-tile index
            o_t = outp.tile([P, E], BF16, tag="o")
            po = psB.tile([P, 2 * QG], F32, tag="pair", name="po")
            nc.tensor.matmul(po[:, 0:QG],
                             num_sb[0:D, t * P:(t + 1) * P],
                             wv2_sb[0:D, 0:QG],
                             start=True, stop=True)
            nc.tensor.matmul(po[:, QG:2 * QG],
                             num_sb[D:P, t * P:(t + 1) * P],
                             wv2_sb[D:P, QG:2 * QG],
                             start=True, stop=True)
            if split:
                # tail: halve the drain latency across ACT+DVE and ship
                # each half as soon as it lands
                nc.scalar.activation(o_t[:, 0:QG], po[:, 0:QG], COPY_FN,
                                     scale=recip[:, t:t + 1])
                nc.vector.tensor_scalar_mul(o_t[:, QG:E], po[:, QG:2 * QG],
                                            recip[:, t:t + 1])
                hw_engs[t % 2].dma_start(out[i * P:(i + 1) * P, 0:QG],
                                         o_t[:, 0:QG])
                hw_engs[(t + 1) % 2].dma_start(out[i * P:(i + 1) * P, QG:E],
                                               o_t[:, QG:E])
            else:
                if drain == "v":
                    nc.vector.tensor_scalar_mul(o_t[:, :], po[:, :],
                                                recip[:, t:t + 1])
                else:
                    nc.scalar.activation(o_t[:, :], po[:, :], COPY_FN,
                                         scale=recip[:, t:t + 1])
                hw_engs[t % 2].dma_start(out[i * P:(i + 1) * P, :], o_t[:, :])

        # Software-pipelined schedule: strips for group g+1 are produced one
        # full period ahead, so the numerator matmuls of period g always read
        # exp'd data - TensorE never waits on ACT latency.
        proj_pass_qk(0)
        # defer groups 2-3 (not needed until ~period 1): a WAW byte-dep
        # on qkt (written when proj(0) drains) holds these back so the
        # contended early HBM window carries only groups 0-1.
        nc.vector.tensor_copy(xT[0:1, 2, 0, 0:1], qkt_sb[0:1, 0:1])
        nc.vector.tensor_copy(xT[0:1, 3, 0, 0:1], qkt_sb[0:1, 0:1])
        nc.sync.dma_start(xT[:, 2], xtv[:, 2])
        nc.sync.dma_start(xT[:, 3], xtv[:, 3])
        # filler over the proj(0)-drain -> kt/qt_hi -> strips(0) latency
        # chain: the 0.8-1.3us gaps there measured enough to trip the
        # HAM MID window at ~22us (3.4us cold follows otherwise)
        emit_warm(5)
        # consts after the first QK pass: keeps the gpsimd queue clear so
        # the kt(0)/qt_hi(0) re-base DMAs land right behind the weights
        make_identity(nc, ident[:, :])
        make_identity(nc, ident4[:, :])
        make_upper_triangular(nc, tri[:, :], val=1.0, diag=True)
        proj_pass_v1(0)
        vp_transp(0)
        emit_warm(4)
        # Trailing numerator for the LAST group: its strip->numerator
        # matmuls are emitted during period 2 (lagging the strip stream so
        # they never wait on exp), leaving only the final few for the
        # drain-limited last period.
        LAG = 6
        tail_num = {"psn": None, "done": 0}

        def num_tail_advance(ents, upto):
            upto = min(upto, len(ents))
            if tail_num["done"] >= upto:
                return
            if tail_num["psn"] is None:
                tail_num["psn"] = psN.tile([D + 1, QG], F32, tag="psn",
                                           name="psn3")
            psn3 = tail_num["psn"]
            for (j, pt_ap, lo) in ents[tail_num["done"]:upto]:
                nc.tensor.matmul(
                    psn3[:, lo:QG], vp_sb[:, j, :], pt_ap,
                    start=(j == 0), stop=(j == NS - 1))
            tail_num["done"] = upto

        entries = []
        for j in range(0, GT, 2):
            entries.extend(emit_strip_pair(0, j))
        nr = {}
        for g in range(NQG - 1):
            n_st = (g + 1) * GT
            items = []
            ng = g + 1

            def out_item(t, idx):
                pn, pr = nr[g - 1]
                # mid-phase drains stay on DVE: an ACT drain sits in the
                # FIFO ahead of the next exp and delays the strip stream
                items.insert(idx, lambda t=t, pn=pn, pr=pr, g2=g - 1:
                             out_pair(g2, t, pn, pr, drain="v"))

            # psP users (proj parts, transposes, keep-alive) stay
            # contiguous in emission order modulo non-psP items, so the
            # two-part accumulating passes are never broken by another
            # psP slot allocation mid-pass.
            items.append(lambda ng=ng: proj_pass_qk(ng, 0))
            items.append(lambda ng=ng: proj_pass_qk(ng, 1))
            items.append(lambda ng=ng: proj_pass_v1(ng, 0))
            items.append(lambda ng=ng: proj_pass_v1(ng, 1))
            items.append(lambda ng=ng: vp_transp(ng, 0))
            items.append(lambda ng=ng: vp_transp(ng, 1))
            if g - 1 >= 0:
                # interleave the out pairs between the pass parts
                out_item(0, 1)
                out_item(1, 3)
                out_item(2, 5)
                out_item(3, 7)
            next_entries = []
            for j in range(0, (g + 2) * GT, 2):
                if ng == NQG - 1:
                    items.append(
                        lambda j=j, g2=ng, acc=next_entries:
                        (acc.extend(emit_strip_pair(g2, j)),
                         num_tail_advance(acc, len(acc) - LAG)))
                else:
                    items.append(
                        lambda j=j, g2=ng, acc=next_entries:
                        acc.extend(emit_strip_pair(g2, j)))
            # trailing keep-alive so the HAM MID window never sees an
            # idle PE while the period's consumers (exp/drains) catch up
            items.append(lambda: emit_warm(2))
            psn = psN.tile([D + 1, QG], F32, tag="psn")
            ii = 0
            for (j, pt_ap, lo) in entries:
                nc.tensor.matmul(
                    psn[:, lo:QG], vp_sb[:, j, :], pt_ap,
                    start=(j == 0), stop=(j == n_st - 1))
                if ii < len(items):
                    items[ii]()
                    ii += 1
            while ii < len(items):
                items[ii]()
                ii += 1
            nr[g] = emit_epilogue(g, psn)
            entries = next_entries
        # final period: drain the remaining trailing numerator MMs with the
        # previous group's out pairs interleaved, then the covered epilogue.
        # Keep-alive dummies between pairs hold the HAM clock gate at 8/8.
        pn, pr = nr[NQG - 2]
        num_tail_advance(entries, NS - 4)
        out_pair(NQG - 2, 0, pn, pr, split=True)
        num_tail_advance(entries, NS - 2)
        out_pair(NQG - 2, 1, pn, pr, split=True)
        num_tail_advance(entries, NS)
        # final epilogue first - its DMA round trip is the tail's critical
        # path; the remaining out(2) pairs execute under it as real cover
        nr[NQG - 1] = emit_epilogue(NQG - 1, tail_num["psn"], cover=True)
        out_pair(NQG - 2, 2, pn, pr, split=True)
        out_pair(NQG - 2, 3, pn, pr, split=True)
        num_sb, recip = nr[NQG - 1]
        for t in range(GT):
            out_pair(NQG - 1, t, num_sb, recip, split=True)

_CACHE = {}


def _get_compiled():
    if "nc" not in _CACHE:
        nc = bacc.Bacc("TRN2", target_bir_lowering=False, debug=False,
                       enable_asserts=False, num_devices=B)
        build_kernel(nc)
        nc.compile()
        _CACHE["nc"] = nc
    return _CACHE["nc"]


def _prep_w(w):
    """[E, M] -> pre-tiled [128, NE*M] bf16 with w'[p, c*M+m] = w[c*128+p, m]."""
    w = np.asarray(w, dtype=np.float32)
    m = w.shape[1]
    return np.ascontiguousarray(
        w.reshape(NE, P, m).transpose(1, 0, 2).reshape(P, NE * m)
        .astype(ml_dtypes.bfloat16))


def _prep_x(x1):
    """[S, E] -> [128, NQG*NE*QG] bf16, x'[p, g*NE*QG + c*QG + s] =
    x[g*QG+s, c*128+p]."""
    return np.ascontiguousarray(
        x1.reshape(NQG, QG, NE, P).transpose(3, 0, 2, 1).reshape(P, -1)
        .astype(ml_dtypes.bfloat16))


def _run(inputs, trace=False, tmpdir=None):
    nc = _get_compiled()
    bf16 = ml_dtypes.bfloat16
    x = np.asarray(inputs["x"], dtype=np.float32)
    wqk = np.concatenate(
        [np.asarray(inputs["w_q"], dtype=np.float32) * SCALE,
         np.asarray(inputs["w_k"], dtype=np.float32)], axis=1)  # [E, 128]
    w = {
        "w_qk": _prep_w(wqk),
        "w_v1": _prep_w(np.asarray(inputs["w_v1"], dtype=np.float32)),
        "w_v2": np.ascontiguousarray(
            np.asarray(inputs["w_v2"], dtype=np.float32).astype(bf16)),
    }
    in_maps = [dict(x_t=_prep_x(x[i]), **w) for i in range(B)]
    res = bass_utils.run_bass_kernel_spmd(
        nc, in_maps, core_ids=list(range(B)), trace=trace, tmpdir=tmpdir,
    )
    outs = np.stack([np.asarray(res.results[i]["out"]) for i in range(B)])
    return outs.astype(np.float32), res


def kernel(**inputs) -> np.ndarray:
    outs, _ = _run(inputs, trace=False)
    return outs


# revision 24
# speedup vs baseline: 1.0880x; 1.0880x over previous
"""Trainium2 Bass kernel: single-head causal attention, data-parallel over batch.

Per core (one batch element):
    Q = x @ w_q; K = x @ w_k; V = (x @ w_v1) @ w_v2
    out = softmax_causal(Q K^T / sqrt(64)) @ V

Sharding: batch 8 -> one element per NeuronCore, weights replicated.

Design notes (v2 of this kernel; hardware-measured 76.6us vs 84.5us v1):
- Low-rank reassociation: attn @ V = (attn @ Vp) @ w_v2 (rank 64), so the
  numerator GEMM contracts to width 64 instead of 1024.
- Scores computed transposed (S^T = K Q^T) so P^T = exp(S^T) lands in the
  lhsT layout of the numerator matmul; a ones column on Vp makes row 64
  of the numerator the softmax denominator for free.
- PE-array ROW TILING for the K=64 matmuls: score strips are emitted as
  concurrent pairs - even strip in array rows 0-63 (kt_sb + Q^T at
  partitions 0-63), odd strip in rows 64-127 (K^T read in place from
  qkt_sb[64:128] + a re-based Q^T copy at partitions 64-127). Verified on
  HW: the pair's matmuls start ~6ns apart when no semaphore wait blocks
  the second. The pair lands in one [128,1024] PSUM tile spanning two
  banks, so a single ACT exp drains both strips (ACT instruction count
  nearly halves - ACT is the mid-phase critical engine).
- Out-GEMM (K=64) row-paired the same way: the two E-halves of a q-tile
  run in rows 0-63 / 64-127 against partition-replicated num / w_v2 into
  one [128,1024] pair, drained by ONE tensor_scalar with recip fused.
- PSUM: one double-buffered 2-bank pair pool shared by score and out
  pairs. bufs=1 self-chained producer->consumer->producer and left the
  PE idle long enough for the HAM clock gate to re-throttle to 1.2GHz
  (the dominant failure mode of every slower variant of this kernel).
- kt/qt_hi re-base copies ride the HWDGE queues (the SWDGE hop measured
  ~4us latency and stalled the first strip pairs of each group).
- Head: groups 2-3 deferred behind a WAW byte-dep on qkt so the early
  HBM window (8 cores contending, per-queue ~130GB/s) carries only
  groups 0-1; 36 N=256 warmup matmuls cover DMA-wait so the clock gate
  is at 8/8 when real work starts. N=64 fillers do NOT register as HAM
  activity - only chunky N>=256 matmuls hold the gate.
- Tail: the final epilogue's d4 round trip rides the sync HWDGE queue
  and is bridged by data-dependent cover matmuls; drains split ACT+DVE.
- Output written bf16 (host upcasts); well inside tolerance.
"""

import os
import sys

import numpy as np

for _p in ("/opt/trn_rl_repo", "/root/.axon_site/_ro/trn_rl_repo"):
    if os.path.isdir(_p) and _p not in sys.path:
        sys.path.insert(0, _p)
os.environ.setdefault("MYCRO_LOCAL_CACHE", "1")

import ml_dtypes  # noqa: E402
import concourse.bass as bass  # noqa: E402
import concourse.mybir as mybir  # noqa: E402
import concourse.tile as tile  # noqa: E402
from concourse import bacc  # noqa: E402
from concourse import bass_utils  # noqa: E402
from concourse.masks import make_identity, make_upper_triangular  # noqa: E402

F32 = mybir.dt.float32
BF16 = mybir.dt.bfloat16

B, S, E, D = 8, 2048, 1024, 64
P = 128
NS = S // P       # 16 s/q tiles
NE = E // P       # 8 E-chunks (projection contraction)
QG = 512          # q-group width
NQG = S // QG     # 4 q-groups
GT = QG // P      # 4 q-tiles per group
SCALE = D ** -0.5
EXP_FN = mybir.ActivationFunctionType.Exp
COPY_FN = mybir.ActivationFunctionType.Copy
NWARM = 36        # dummy matmuls to warm the PE clock gate during loads


def build_kernel(nc):
    # x pre-tiled on host: x_t[p, g*NE*QG + c*QG + s] = x[g*QG+s, c*128+p]
    x_t = nc.dram_tensor("x_t", (P, NQG * NE * QG), BF16,
                         kind="ExternalInput").ap()
    # w_qk pre-tiled: w_qk[p, c*128 + m] = [w_q*scale | w_k][c*128+p, m]
    w_qk = nc.dram_tensor("w_qk", (P, NE * P), BF16, kind="ExternalInput").ap()
    w_v1 = nc.dram_tensor("w_v1", (P, NE * D), BF16, kind="ExternalInput").ap()
    w_v2 = nc.dram_tensor("w_v2", (D, E), BF16, kind="ExternalInput").ap()
    out = nc.dram_tensor("out", (S, E), BF16, kind="ExternalOutput").ap()

    with tile.TileContext(nc) as tc:
        _body(tc, nc, x_t, w_qk, w_v1, w_v2, out)


def _body(tc, nc, x_t, w_qk, w_v1, w_v2, out):
    from contextlib import ExitStack

    with ExitStack() as ctx:
        const = ctx.enter_context(tc.tile_pool(name="const", bufs=1))
        big = ctx.enter_context(tc.tile_pool(name="big", bufs=1))
        # pt pairs live from exp until their last numerator read; sized
        # above peak-live so pool-reuse WARs never stall the producers
        ptp = ctx.enter_context(tc.tile_pool(name="ptp", bufs=14))
        outp = ctx.enter_context(tc.tile_pool(name="outp", bufs=6))
        small = ctx.enter_context(tc.tile_pool(name="small", bufs=8))
        # PSUM budget (8 banks): one double-buffered 2-bank pair pool
        # shared by score pairs AND out pairs (4) + psP 2x1 + psN 2x1.
        # Two slots mean a pair's consumer (exp / drain) never gates the
        # NEXT pair's matmuls - the engines each keep a backlog.
        psB = ctx.enter_context(tc.tile_pool(name="psB", bufs=2, space="PSUM"))
        psP = ctx.enter_context(tc.tile_pool(name="psP", bufs=2, space="PSUM"))
        psN = ctx.enter_context(tc.tile_pool(name="psN", bufs=2, space="PSUM"))

        # ---- warm-up operand: memset immediately, no DMA dependency ----
        wu = const.tile([P, 256], BF16, tag="wu")
        nc.vector.memset(wu[:, :], 0.001)

        # ---- weight + x^T loads ----
        xT = big.tile([P, NQG, NE, QG], BF16, tag="xT")
        xtv = x_t.rearrange("p (g c s) -> p g c s", g=NQG, c=NE)
        wqk_sb = const.tile([P, NE, P], BF16, tag="wqk")
        wv1_sb = const.tile([P, NE, D], BF16, tag="wv1")
        # w_v2 replicated into both partition halves: row-paired out GEMMs
        # read rhs at partitions 0-63 (eh=0) and 64-127 (eh=1)
        wv2_sb = const.tile([P, E], BF16, tag="wv2")
        # wqk split across both HWDGE queues so x group 0 doesn't wait
        # behind the full weight transfer
        wqkv = w_qk.rearrange("p (c m) -> p c m", m=P)
        h = NE // 2
        nc.sync.dma_start(wqk_sb[:, 0:h, :], wqkv[:, 0:h, :])
        nc.scalar.dma_start(wqk_sb[:, h:NE, :], wqkv[:, h:NE, :])
        # x group 0 as one half per HWDGE queue (4KB/partition lines)
        nc.scalar.dma_start(xT[:, 0, 0:h, :], xtv[:, 0, 0:h, :])
        nc.sync.dma_start(xT[:, 0, h:NE, :], xtv[:, 0, h:NE, :])
        hw_engs = (nc.sync, nc.scalar)
        # group 1 next on both queues - needed early (proj(1) in period 0)
        nc.scalar.dma_start(xT[:, 1, 0:h, :], xtv[:, 1, 0:h, :])
        nc.sync.dma_start(xT[:, 1, h:NE, :], xtv[:, 1, h:NE, :])
        nc.gpsimd.dma_start(wv1_sb[:, :, :],
                            w_v1.rearrange("p (c d) -> p c d", d=D))
        nc.gpsimd.dma_start(wv2_sb[0:D, :], w_v2)
        nc.gpsimd.dma_start(wv2_sb[D:P, :], w_v2)

        ident = const.tile([D, D], BF16, tag="ident")
        ident4 = const.tile([GT, GT], F32, tag="ident4")
        tri = const.tile([P, P], BF16, tag="tri")
        ones1 = const.tile([1, D], F32, tag="ones1")
        nc.vector.memset(ones1[:, :], 1.0)
        # tri[s, q] = 1 where s <= q else 0 (valid causal region, S^T layout)

        def emit_warm(n, width=256):
            """Dummy matmuls to keep the PE clock gate at 8/8."""
            psw = None
            for _ in range(n):
                psw = psP.tile([P, 256], F32, tag="psP")
                nc.tensor.matmul(psw[:, 0:width], wu[:, 0:P],
                                 wu[:, 0:width], start=True, stop=True)
            # token reader so the verifier sees the results consumed
            nc.vector.tensor_copy(wu[0:1, 0:1], psw[0:1, 0:1])

        # ---- PE warm-up: dummy matmuls while DMAs stream in ----
        # N=64 fillers were tried and do NOT register as busy in the HAM
        # activity window (50% duty cycle reads as idle); only chunky
        # N=256 back-to-back matmuls hold the clock gate.
        emit_warm(NWARM)

        qkt_sb = big.tile([P, S], BF16, tag="qkt")
        kt_sb = big.tile([D, S], BF16, tag="kt")
        # Q^T re-based to partitions 64-127 (rhs of odd score strips)
        qt_hi = big.tile([P, S], BF16, tag="qthi")
        vpt_sb = big.tile([D, S], BF16, tag="vpt")
        # Vp tile-wise as [s, 64+1] (numerator lhsT); ones column -> denom row
        vp_sb = big.tile([P, NS, D + 1], BF16, tag="vp")
        nc.vector.memset(vp_sb[:, :, D], 1.0)

        proj_ps = {}

        def proj_pass_qk(ng, part=None):
            """part=0/1 emits half the chunks (finer PE interleave
            granularity); part=None emits the whole pass."""
            sl = slice(ng * QG, (ng + 1) * QG)
            if part != 1:
                proj_ps["qk", ng] = psP.tile([P, QG], F32, tag="psP", name="psqk")
            ps = proj_ps["qk", ng]
            lo = 0 if part != 1 else NE // 2
            hi = NE // 2 if part == 0 else NE
            for ec in range(lo, hi):
                nc.tensor.matmul(
                    ps[:, :], wqk_sb[:, ec, :], xT[:, ng, ec, :],
                    start=(ec == 0), stop=(ec == NE - 1))
            if part == 0:
                return
            nc.vector.tensor_copy(qkt_sb[:, sl], ps[:, :])
            # strip pair operands: K^T at partitions 0-63 (even strips),
            # Q^T at partitions 64-127 (odd strips). HWDGE queues: the
            # SWDGE hop measured ~4us latency and stalled the first strips
            nc.sync.dma_start(kt_sb[:, sl], qkt_sb[D:P, sl])
            nc.scalar.dma_start(qt_hi[D:P, sl], qkt_sb[0:D, sl])

        def proj_pass_v1(ng, part=None):
            sl = slice(ng * QG, (ng + 1) * QG)
            if part != 1:
                proj_ps["v1", ng] = psP.tile([P, QG], F32, tag="psP", name="psv1")
            ps = proj_ps["v1", ng]
            lo = 0 if part != 1 else NE // 2
            hi = NE // 2 if part == 0 else NE
            for ec in range(lo, hi):
                nc.tensor.matmul(
                    ps[0:D, :], wv1_sb[:, ec, :], xT[:, ng, ec, :],
                    start=(ec == 0), stop=(ec == NE - 1))
            if part == 0:
                return
            nc.scalar.copy(vpt_sb[:, sl], ps[0:D, :])

        def vp_transp(ng, part=None):
            lo = ng * GT + (2 if part == 1 else 0)
            hi = ng * GT + (2 if part == 0 else GT)
            for st in range(lo, hi):
                pst = psP.tile([P, D], BF16, tag="psP")
                nc.tensor.transpose(pst[0:P, 0:D],
                                    vpt_sb[:, st * P:(st + 1) * P],
                                    ident[:, :])
                nc.vector.tensor_copy(vp_sb[:, st, 0:D], pst[0:P, 0:D])

        def _lo(qg, j):
            dt_blk = j - qg * GT
            return dt_blk * P if 0 < dt_blk < GT else 0

        def emit_strip_pair(qg, j):
            """Strips j (even) and j+1 as a concurrent row-tiled pair in
            one 2-bank PSUM tile, drained by a single fused exp when the
            written region is contiguous."""
            lo0, lo1 = _lo(qg, j), _lo(qg, j + 1)
            ps = psB.tile([P, 2 * QG], F32, tag="pair")
            # even strip -> array rows 0-63
            nc.tensor.matmul(
                ps[:, lo0:QG],
                kt_sb[:, j * P:(j + 1) * P],
                qkt_sb[0:D, qg * QG + lo0:(qg + 1) * QG],
                start=True, stop=True,
            )
            # odd strip -> array rows 64-127 (K^T in place, Q^T replica)
            nc.tensor.matmul(
                ps[:, QG + lo1:2 * QG],
                qkt_sb[D:P, (j + 1) * P:(j + 2) * P],
                qt_hi[D:P, qg * QG + lo1:(qg + 1) * QG],
                start=True, stop=True,
            )
            pt = ptp.tile([P, 2 * QG], BF16, tag="pt")
            if lo0 == 0 and lo1 == 0:
                nc.scalar.activation(pt[:, :], ps[:, :], EXP_FN)
            else:
                nc.scalar.activation(pt[:, lo0:QG], ps[:, lo0:QG], EXP_FN)
                nc.scalar.activation(pt[:, QG + lo1:], ps[:, QG + lo1:],
                                     EXP_FN)
            out = []
            for jj, off, lo in ((j, 0, lo0), (j + 1, QG, lo1)):
                dt_blk = jj - qg * GT
                if 0 <= dt_blk < GT:
                    # mask the diagonal 128x128 block (cols < lo are never
                    # read: numerator MMs are lo-trimmed)
                    nc.gpsimd.tensor_mul(
                        pt[:, off + dt_blk * P:off + (dt_blk + 1) * P],
                        pt[:, off + dt_blk * P:off + (dt_blk + 1) * P],
                        tri[:, :],
                    )
                out.append((jj, pt[:, off + lo:off + QG], lo))
            return out

        def emit_epilogue(qg, psn, cover=False):
            """Denominator row -> per-partition recip; numerator -> bf16
            replicated to both partition halves (row-paired out GEMM).
            cover=True: bridge the d4 DMA round trip with dummy matmuls
            that READ d_sb so the scheduler cannot hoist them."""
            d_sb = small.tile([1, QG], F32, tag="dsb")
            if cover:  # tail: ACT is free; DVE still drains out tiles
                nc.scalar.copy(d_sb[:, :], psn[D:D + 1, :])
            else:
                nc.vector.tensor_copy(d_sb[:, :], psn[D:D + 1, :])
            d4 = small.tile([GT, P], F32, tag="d4")
            # tail: SWDGE hop is ~4us latency; sync HWDGE is fast there
            (nc.sync if cover else nc.gpsimd).dma_start(d4[:, :],
                                                        d_sb[0:1, :])
            if cover:
                psw = None
                for _ in range(3):
                    psw = psP.tile([P, 256], F32, tag="psP")
                    nc.tensor.matmul(psw[0:D, :], ones1[:, :],
                                     d_sb[0:1, 0:256],
                                     start=True, stop=True)
                nc.vector.tensor_copy(d_sb[0:1, 0:1], psw[0:1, 0:1])
            ps4 = psP.tile([P, GT], F32, tag="psP")
            nc.tensor.transpose(ps4[0:P, 0:GT], d4[:, :], ident4[:, :])
            recip = small.tile([P, GT], F32, tag="recip")
            nc.vector.reciprocal(recip[:, :], ps4[0:P, 0:GT])
            num_sb = small.tile([P, QG], BF16, tag="numsb")
            if cover:
                # tail critical path: produce both halves immediately on
                # the two drain engines in parallel
                nc.vector.tensor_copy(num_sb[0:D, :], psn[0:D, :])
                nc.scalar.copy(num_sb[D:P, :], psn[0:D, :])
            else:
                nc.vector.tensor_copy(num_sb[0:D, :], psn[0:D, :])
                # replica to partitions 64-127 rides the SWDGE queue
                nc.gpsimd.dma_start(num_sb[D:P, :], num_sb[0:D, :])
            return num_sb, recip

        def out_pair(qg, t, num_sb, recip, drain="v", split=False):
            """One q-tile's two E-halves as a concurrent row-tiled pair,
            drained by a single fused tensor_scalar (recip folded in)."""
            i = qg * GT + t  # global q-tile index
            o_t = outp.tile([P, E], BF16, tag="o")
            po = psB.tile([P, 2 * QG], F32, tag="pair", name="po")
            nc.tensor.matmul(po[:, 0:QG],
                             num_sb[0:D, t * P:(t + 1) * P],
                             wv2_sb[0:D, 0:QG],
                             start=True, stop=True)
            nc.tensor.matmul(po[:, QG:2 * QG],
                             num_sb[D:P, t * P:(t + 1) * P],
                             wv2_sb[D:P, QG:2 * QG],
                             start=True, stop=True)
            if split:
                # tail: halve the drain latency across ACT+DVE and ship
                # each half as soon as it lands
                nc.scalar.activation(o_t[:, 0:QG], po[:, 0:QG], COPY_FN,
                                     scale=recip[:, t:t + 1])
                nc.vector.tensor_scalar_mul(o_t[:, QG:E], po[:, QG:2 * QG],
                                            recip[:, t:t + 1])
                hw_engs[t % 2].dma_start(out[i * P:(i + 1) * P, 0:QG],
                                         o_t[:, 0:QG])
                hw_engs[(t + 1) % 2].dma_start(out[i * P:(i + 1) * P, QG:E],
                                               o_t[:, QG:E])
            else:
                if drain == "v":
                    nc.vector.tensor_scalar_mul(o_t[:, :], po[:, :],
                                                recip[:, t:t + 1])
                else:
                    nc.scalar.activation(o_t[:, :], po[:, :], COPY_FN,
                                         scale=recip[:, t:t + 1])
                hw_engs[t % 2].dma_start(out[i * P:(i + 1) * P, :], o_t[:, :])

        # Software-pipelined schedule: strips for group g+1 are produced one
        # full period ahead, so the numerator matmuls of period g always read
        # exp'd data - TensorE never waits on ACT latency.
        proj_pass_qk(0)
        # defer groups 2-3 (not needed until ~period 1): a WAW byte-dep
        # on qkt (written when proj(0) drains) holds these back so the
        # contended early HBM window carries only groups 0-1.
        nc.vector.tensor_copy(xT[0:1, 2, 0, 0:1], qkt_sb[0:1, 0:1])
        nc.vector.tensor_copy(xT[0:1, 3, 0, 0:1], qkt_sb[0:1, 0:1])
        nc.sync.dma_start(xT[:, 2], xtv[:, 2])
        nc.sync.dma_start(xT[:, 3], xtv[:, 3])
        # consts after the first QK pass: keeps the gpsimd queue clear so
        # the kt(0)/qt_hi(0) re-base DMAs land right behind the weights
        make_identity(nc, ident[:, :])
        make_identity(nc, ident4[:, :])
        make_upper_triangular(nc, tri[:, :], val=1.0, diag=True)
        proj_pass_v1(0)
        vp_transp(0)
        # Trailing numerator for the LAST group: its strip->numerator
        # matmuls are emitted during period 2 (lagging the strip stream so
        # they never wait on exp), leaving only the final few for the
        # drain-limited last period.
        LAG = 6
        tail_num = {"psn": None, "done": 0}

        def num_tail_advance(ents, upto):
            upto = min(upto, len(ents))
            if tail_num["done"] >= upto:
                return
            if tail_num["psn"] is None:
                tail_num["psn"] = psN.tile([D + 1, QG], F32, tag="psn",
                                           name="psn3")
            psn3 = tail_num["psn"]
            for (j, pt_ap, lo) in ents[tail_num["done"]:upto]:
                nc.tensor.matmul(
                    psn3[:, lo:QG], vp_sb[:, j, :], pt_ap,
                    start=(j == 0), stop=(j == NS - 1))
            tail_num["done"] = upto

        entries = []
        for j in range(0, GT, 2):
            entries.extend(emit_strip_pair(0, j))
        nr = {}
        for g in range(NQG - 1):
            n_st = (g + 1) * GT
            items = []
            ng = g + 1

            def out_item(t, idx):
                pn, pr = nr[g - 1]
                # mid-phase drains stay on DVE: an ACT drain sits in the
                # FIFO ahead of the next exp and delays the strip stream
                items.insert(idx, lambda t=t, pn=pn, pr=pr, g2=g - 1:
                             out_pair(g2, t, pn, pr, drain="v"))

            # psP users (proj parts, transposes, keep-alive) stay
            # contiguous in emission order modulo non-psP items, so the
            # two-part accumulating passes are never broken by another
            # psP slot allocation mid-pass.
            items.append(lambda ng=ng: proj_pass_qk(ng, 0))
            items.append(lambda ng=ng: proj_pass_qk(ng, 1))
            items.append(lambda ng=ng: proj_pass_v1(ng, 0))
            items.append(lambda ng=ng: proj_pass_v1(ng, 1))
            items.append(lambda ng=ng: vp_transp(ng, 0))
            items.append(lambda ng=ng: vp_transp(ng, 1))
            if g - 1 >= 0:
                # interleave the out pairs between the pass parts
                out_item(0, 1)
                out_item(1, 3)
                out_item(2, 5)
                out_item(3, 7)
            next_entries = []
            for j in range(0, (g + 2) * GT, 2):
                if ng == NQG - 1:
                    items.append(
                        lambda j=j, g2=ng, acc=next_entries:
                        (acc.extend(emit_strip_pair(g2, j)),
                         num_tail_advance(acc, len(acc) - LAG)))
                else:
                    items.append(
                        lambda j=j, g2=ng, acc=next_entries:
                        acc.extend(emit_strip_pair(g2, j)))
            # trailing keep-alive so the HAM MID window never sees an
            # idle PE while the period's consumers (exp/drains) catch up
            items.append(lambda: emit_warm(2))
            psn = psN.tile([D + 1, QG], F32, tag="psn")
            ii = 0
            for (j, pt_ap, lo) in entries:
                nc.tensor.matmul(
                    psn[:, lo:QG], vp_sb[:, j, :], pt_ap,
                    start=(j == 0), stop=(j == n_st - 1))
                if ii < len(items):
                    items[ii]()
                    ii += 1
            while ii < len(items):
                items[ii]()
                ii += 1
            nr[g] = emit_epilogue(g, psn)
            entries = next_entries
        # final period: drain the remaining trailing numerator MMs with the
        # previous group's out pairs interleaved, then the covered epilogue.
        # Keep-alive dummies between pairs hold the HAM clock gate at 8/8.
        pn, pr = nr[NQG - 2]
        num_tail_advance(entries, NS - 4)
        out_pair(NQG - 2, 0, pn, pr, split=True)
        num_tail_advance(entries, NS - 2)
        out_pair(NQG - 2, 1, pn, pr, split=True)
        num_tail_advance(entries, NS)
        # final epilogue first - its DMA round trip is the tail's critical
        # path; the remaining out(2) pairs execute under it as real cover
        nr[NQG - 1] = emit_epilogue(NQG - 1, tail_num["psn"], cover=True)
        out_pair(NQG - 2, 2, pn, pr, split=True)
        out_pair(NQG - 2, 3, pn, pr, split=True)
        num_sb, recip = nr[NQG - 1]
        for t in range(GT):
            out_pair(NQG - 1, t, num_sb, recip, split=True)

_CACHE = {}


def _get_compiled():
    if "nc" not in _CACHE:
        nc = bacc.Bacc("TRN2", target_bir_lowering=False, debug=False,
                       enable_asserts=False, num_devices=B)
        build_kernel(nc)
        nc.compile()
        _CACHE["nc"] = nc
    return _CACHE["nc"]


def _prep_w(w):
    """[E, M] -> pre-tiled [128, NE*M] bf16 with w'[p, c*M+m] = w[c*128+p, m]."""
    w = np.asarray(w, dtype=np.float32)
    m = w.shape[1]
    return np.ascontiguousarray(
        w.reshape(NE, P, m).transpose(1, 0, 2).reshape(P, NE * m)
        .astype(ml_dtypes.bfloat16))


def _prep_x(x1):
    """[S, E] -> [128, NQG*NE*QG] bf16, x'[p, g*NE*QG + c*QG + s] =
    x[g*QG+s, c*128+p]."""
    return np.ascontiguousarray(
        x1.reshape(NQG, QG, NE, P).transpose(3, 0, 2, 1).reshape(P, -1)
        .astype(ml_dtypes.bfloat16))


def _run(inputs, trace=False, tmpdir=None):
    nc = _get_compiled()
    bf16 = ml_dtypes.bfloat16
    x = np.asarray(inputs["x"], dtype=np.float32)
    wqk = np.concatenate(
        [np.asarray(inputs["w_q"], dtype=np.float32) * SCALE,
         np.asarray(inputs["w_k"], dtype=np.float32)], axis=1)  # [E, 128]
    w = {
        "w_qk": _prep_w(wqk),
        "w_v1": _prep_w(np.asarray(inputs["w_v1"], dtype=np.float32)),
        "w_v2": np.ascontiguousarray(
            np.asarray(inputs["w_v2"], dtype=np.float32).astype(bf16)),
    }
    in_maps = [dict(x_t=_prep_x(x[i]), **w) for i in range(B)]
    res = bass_utils.run_bass_kernel_spmd(
        nc, in_maps, core_ids=list(range(B)), trace=trace, tmpdir=tmpdir,
    )
    outs = np.stack([np.asarray(res.results[i]["out"]) for i in range(B)])
    return outs.astype(np.float32), res


def kernel(**inputs) -> np.ndarray:
    outs, _ = _run(inputs, trace=False)
    return outs


# revision 25
# speedup vs baseline: 1.0923x; 1.0039x over previous
"""Trainium2 Bass kernel: single-head causal attention, data-parallel over batch.

Per core (one batch element):
    Q = x @ w_q; K = x @ w_k; V = (x @ w_v1) @ w_v2
    out = softmax_causal(Q K^T / sqrt(64)) @ V

Sharding: batch 8 -> one element per NeuronCore, weights replicated.

Design notes (v2 of this kernel; hardware-measured 76.6us vs 84.5us v1):
- Low-rank reassociation: attn @ V = (attn @ Vp) @ w_v2 (rank 64), so the
  numerator GEMM contracts to width 64 instead of 1024.
- Scores computed transposed (S^T = K Q^T) so P^T = exp(S^T) lands in the
  lhsT layout of the numerator matmul; a ones column on Vp makes row 64
  of the numerator the softmax denominator for free.
- PE-array ROW TILING for the K=64 matmuls: score strips are emitted as
  concurrent pairs - even strip in array rows 0-63 (kt_sb + Q^T at
  partitions 0-63), odd strip in rows 64-127 (K^T read in place from
  qkt_sb[64:128] + a re-based Q^T copy at partitions 64-127). Verified on
  HW: the pair's matmuls start ~6ns apart when no semaphore wait blocks
  the second. The pair lands in one [128,1024] PSUM tile spanning two
  banks, so a single ACT exp drains both strips (ACT instruction count
  nearly halves - ACT is the mid-phase critical engine).
- Out-GEMM (K=64) row-paired the same way: the two E-halves of a q-tile
  run in rows 0-63 / 64-127 against partition-replicated num / w_v2 into
  one [128,1024] pair, drained by ONE tensor_scalar with recip fused.
- PSUM: one double-buffered 2-bank pair pool shared by score and out
  pairs. bufs=1 self-chained producer->consumer->producer and left the
  PE idle long enough for the HAM clock gate to re-throttle to 1.2GHz
  (the dominant failure mode of every slower variant of this kernel).
- kt/qt_hi re-base copies ride the HWDGE queues (the SWDGE hop measured
  ~4us latency and stalled the first strip pairs of each group).
- Head: groups 2-3 deferred behind a WAW byte-dep on qkt so the early
  HBM window (8 cores contending, per-queue ~130GB/s) carries only
  groups 0-1; 36 N=256 warmup matmuls cover DMA-wait so the clock gate
  is at 8/8 when real work starts. N=64 fillers do NOT register as HAM
  activity - only chunky N>=256 matmuls hold the gate.
- Tail: the final epilogue's d4 round trip rides the sync HWDGE queue
  and is bridged by data-dependent cover matmuls; drains split ACT+DVE.
- Output written bf16 (host upcasts); well inside tolerance.
"""

import os
import sys

import numpy as np

for _p in ("/opt/trn_rl_repo", "/root/.axon_site/_ro/trn_rl_repo"):
    if os.path.isdir(_p) and _p not in sys.path:
        sys.path.insert(0, _p)
os.environ.setdefault("MYCRO_LOCAL_CACHE", "1")

import ml_dtypes  # noqa: E402
import concourse.bass as bass  # noqa: E402
import concourse.mybir as mybir  # noqa: E402
import concourse.tile as tile  # noqa: E402
from concourse import bacc  # noqa: E402
from concourse import bass_utils  # noqa: E402
from concourse.masks import make_identity, make_upper_triangular  # noqa: E402

F32 = mybir.dt.float32
BF16 = mybir.dt.bfloat16

B, S, E, D = 8, 2048, 1024, 64
P = 128
NS = S // P       # 16 s/q tiles
NE = E // P       # 8 E-chunks (projection contraction)
QG = 512          # q-group width
NQG = S // QG     # 4 q-groups
GT = QG // P      # 4 q-tiles per group
SCALE = D ** -0.5
EXP_FN = mybir.ActivationFunctionType.Exp
COPY_FN = mybir.ActivationFunctionType.Copy
NWARM = 36        # dummy matmuls to warm the PE clock gate during loads


def build_kernel(nc):
    # x pre-tiled on host: x_t[p, g*NE*QG + c*QG + s] = x[g*QG+s, c*128+p]
    x_t = nc.dram_tensor("x_t", (P, NQG * NE * QG), BF16,
                         kind="ExternalInput").ap()
    # w_qk pre-tiled: w_qk[p, c*128 + m] = [w_q*scale | w_k][c*128+p, m]
    w_qk = nc.dram_tensor("w_qk", (P, NE * P), BF16, kind="ExternalInput").ap()
    w_v1 = nc.dram_tensor("w_v1", (P, NE * D), BF16, kind="ExternalInput").ap()
    w_v2 = nc.dram_tensor("w_v2", (D, E), BF16, kind="ExternalInput").ap()
    out = nc.dram_tensor("out", (S, E), BF16, kind="ExternalOutput").ap()

    with tile.TileContext(nc) as tc:
        _body(tc, nc, x_t, w_qk, w_v1, w_v2, out)


def _body(tc, nc, x_t, w_qk, w_v1, w_v2, out):
    from contextlib import ExitStack

    with ExitStack() as ctx:
        const = ctx.enter_context(tc.tile_pool(name="const", bufs=1))
        big = ctx.enter_context(tc.tile_pool(name="big", bufs=1))
        # pt pairs live from exp until their last numerator read; sized
        # above peak-live so pool-reuse WARs never stall the producers
        ptp = ctx.enter_context(tc.tile_pool(name="ptp", bufs=14))
        outp = ctx.enter_context(tc.tile_pool(name="outp", bufs=6))
        small = ctx.enter_context(tc.tile_pool(name="small", bufs=8))
        # PSUM budget (8 banks): one double-buffered 2-bank pair pool
        # shared by score pairs AND out pairs (4) + psP 2x1 + psN 2x1.
        # Two slots mean a pair's consumer (exp / drain) never gates the
        # NEXT pair's matmuls - the engines each keep a backlog.
        psB = ctx.enter_context(tc.tile_pool(name="psB", bufs=2, space="PSUM"))
        psP = ctx.enter_context(tc.tile_pool(name="psP", bufs=2, space="PSUM"))
        psN = ctx.enter_context(tc.tile_pool(name="psN", bufs=2, space="PSUM"))

        # ---- warm-up operand: memset immediately, no DMA dependency ----
        wu = const.tile([P, 256], BF16, tag="wu")
        nc.vector.memset(wu[:, :], 0.001)

        # ---- weight + x^T loads ----
        xT = big.tile([P, NQG, NE, QG], BF16, tag="xT")
        xtv = x_t.rearrange("p (g c s) -> p g c s", g=NQG, c=NE)
        wqk_sb = const.tile([P, NE, P], BF16, tag="wqk")
        wv1_sb = const.tile([P, NE, D], BF16, tag="wv1")
        # w_v2 replicated into both partition halves: row-paired out GEMMs
        # read rhs at partitions 0-63 (eh=0) and 64-127 (eh=1)
        wv2_sb = const.tile([P, E], BF16, tag="wv2")
        # wqk split across both HWDGE queues so x group 0 doesn't wait
        # behind the full weight transfer
        wqkv = w_qk.rearrange("p (c m) -> p c m", m=P)
        h = NE // 2
        nc.sync.dma_start(wqk_sb[:, 0:h, :], wqkv[:, 0:h, :])
        nc.scalar.dma_start(wqk_sb[:, h:NE, :], wqkv[:, h:NE, :])
        # x group 0 as one half per HWDGE queue (4KB/partition lines)
        nc.scalar.dma_start(xT[:, 0, 0:h, :], xtv[:, 0, 0:h, :])
        nc.sync.dma_start(xT[:, 0, h:NE, :], xtv[:, 0, h:NE, :])
        hw_engs = (nc.sync, nc.scalar)
        # group 1 next on both queues - needed early (proj(1) in period 0)
        nc.scalar.dma_start(xT[:, 1, 0:h, :], xtv[:, 1, 0:h, :])
        nc.sync.dma_start(xT[:, 1, h:NE, :], xtv[:, 1, h:NE, :])
        nc.gpsimd.dma_start(wv1_sb[:, :, :],
                            w_v1.rearrange("p (c d) -> p c d", d=D))
        nc.gpsimd.dma_start(wv2_sb[0:D, :], w_v2)
        nc.gpsimd.dma_start(wv2_sb[D:P, :], w_v2)

        ident = const.tile([D, D], BF16, tag="ident")
        ident4 = const.tile([GT, GT], F32, tag="ident4")
        tri = const.tile([P, P], BF16, tag="tri")
        ones1 = const.tile([1, D], F32, tag="ones1")
        nc.vector.memset(ones1[:, :], 1.0)
        # tri[s, q] = 1 where s <= q else 0 (valid causal region, S^T layout)

        def emit_warm(n, width=256):
            """Dummy matmuls to keep the PE clock gate at 8/8."""
            psw = None
            for _ in range(n):
                psw = psP.tile([P, 256], F32, tag="psP")
                nc.tensor.matmul(psw[:, 0:width], wu[:, 0:P],
                                 wu[:, 0:width], start=True, stop=True)
            # token reader so the verifier sees the results consumed
            nc.vector.tensor_copy(wu[0:1, 0:1], psw[0:1, 0:1])

        # ---- PE warm-up: dummy matmuls while DMAs stream in ----
        # N=64 fillers were tried and do NOT register as busy in the HAM
        # activity window (50% duty cycle reads as idle); only chunky
        # N=256 back-to-back matmuls hold the clock gate.
        emit_warm(NWARM)

        qkt_sb = big.tile([P, S], BF16, tag="qkt")
        kt_sb = big.tile([D, S], BF16, tag="kt")
        # Q^T re-based to partitions 64-127 (rhs of odd score strips)
        qt_hi = big.tile([P, S], BF16, tag="qthi")
        vpt_sb = big.tile([D, S], BF16, tag="vpt")
        # Vp tile-wise as [s, 64+1] (numerator lhsT); ones column -> denom row
        vp_sb = big.tile([P, NS, D + 1], BF16, tag="vp")
        nc.vector.memset(vp_sb[:, :, D], 1.0)

        proj_ps = {}

        def proj_pass_qk(ng, part=None):
            """part=0/1 emits half the chunks (finer PE interleave
            granularity); part=None emits the whole pass."""
            sl = slice(ng * QG, (ng + 1) * QG)
            if part != 1:
                proj_ps["qk", ng] = psP.tile([P, QG], F32, tag="psP", name="psqk")
            ps = proj_ps["qk", ng]
            lo = 0 if part != 1 else NE // 2
            hi = NE // 2 if part == 0 else NE
            for ec in range(lo, hi):
                nc.tensor.matmul(
                    ps[:, :], wqk_sb[:, ec, :], xT[:, ng, ec, :],
                    start=(ec == 0), stop=(ec == NE - 1))
            if part == 0:
                return
            nc.vector.tensor_copy(qkt_sb[:, sl], ps[:, :])
            # strip pair operands: K^T at partitions 0-63 (even strips),
            # Q^T at partitions 64-127 (odd strips). HWDGE queues: the
            # SWDGE hop measured ~4us latency and stalled the first strips
            nc.sync.dma_start(kt_sb[:, sl], qkt_sb[D:P, sl])
            nc.scalar.dma_start(qt_hi[D:P, sl], qkt_sb[0:D, sl])

        def proj_pass_v1(ng, part=None):
            sl = slice(ng * QG, (ng + 1) * QG)
            if part != 1:
                proj_ps["v1", ng] = psP.tile([P, QG], F32, tag="psP", name="psv1")
            ps = proj_ps["v1", ng]
            lo = 0 if part != 1 else NE // 2
            hi = NE // 2 if part == 0 else NE
            for ec in range(lo, hi):
                nc.tensor.matmul(
                    ps[0:D, :], wv1_sb[:, ec, :], xT[:, ng, ec, :],
                    start=(ec == 0), stop=(ec == NE - 1))
            if part == 0:
                return
            nc.scalar.copy(vpt_sb[:, sl], ps[0:D, :])

        def vp_transp(ng, part=None):
            lo = ng * GT + (2 if part == 1 else 0)
            hi = ng * GT + (2 if part == 0 else GT)
            for st in range(lo, hi):
                pst = psP.tile([P, D], BF16, tag="psP")
                nc.tensor.transpose(pst[0:P, 0:D],
                                    vpt_sb[:, st * P:(st + 1) * P],
                                    ident[:, :])
                nc.vector.tensor_copy(vp_sb[:, st, 0:D], pst[0:P, 0:D])

        def _lo(qg, j):
            dt_blk = j - qg * GT
            return dt_blk * P if 0 < dt_blk < GT else 0

        def emit_strip_pair(qg, j):
            """Strips j (even) and j+1 as a concurrent row-tiled pair in
            one 2-bank PSUM tile, drained by a single fused exp when the
            written region is contiguous."""
            lo0, lo1 = _lo(qg, j), _lo(qg, j + 1)
            ps = psB.tile([P, 2 * QG], F32, tag="pair")
            # even strip -> array rows 0-63
            nc.tensor.matmul(
                ps[:, lo0:QG],
                kt_sb[:, j * P:(j + 1) * P],
                qkt_sb[0:D, qg * QG + lo0:(qg + 1) * QG],
                start=True, stop=True,
            )
            # odd strip -> array rows 64-127 (K^T in place, Q^T replica)
            nc.tensor.matmul(
                ps[:, QG + lo1:2 * QG],
                qkt_sb[D:P, (j + 1) * P:(j + 2) * P],
                qt_hi[D:P, qg * QG + lo1:(qg + 1) * QG],
                start=True, stop=True,
            )
            pt = ptp.tile([P, 2 * QG], BF16, tag="pt")
            if lo0 == 0 and lo1 == 0:
                nc.scalar.activation(pt[:, :], ps[:, :], EXP_FN)
            else:
                nc.scalar.activation(pt[:, lo0:QG], ps[:, lo0:QG], EXP_FN)
                nc.scalar.activation(pt[:, QG + lo1:], ps[:, QG + lo1:],
                                     EXP_FN)
            out = []
            for jj, off, lo in ((j, 0, lo0), (j + 1, QG, lo1)):
                dt_blk = jj - qg * GT
                if 0 <= dt_blk < GT:
                    # mask the diagonal 128x128 block (cols < lo are never
                    # read: numerator MMs are lo-trimmed)
                    nc.gpsimd.tensor_mul(
                        pt[:, off + dt_blk * P:off + (dt_blk + 1) * P],
                        pt[:, off + dt_blk * P:off + (dt_blk + 1) * P],
                        tri[:, :],
                    )
                out.append((jj, pt[:, off + lo:off + QG], lo))
            return out

        def emit_epilogue(qg, psn, cover=False):
            """Denominator row -> per-partition recip; numerator -> bf16
            replicated to both partition halves (row-paired out GEMM).
            cover=True: bridge the d4 DMA round trip with dummy matmuls
            that READ d_sb so the scheduler cannot hoist them."""
            d_sb = small.tile([1, QG], F32, tag="dsb")
            if cover:  # tail: ACT is free; DVE still drains out tiles
                nc.scalar.copy(d_sb[:, :], psn[D:D + 1, :])
            else:
                nc.vector.tensor_copy(d_sb[:, :], psn[D:D + 1, :])
            d4 = small.tile([GT, P], F32, tag="d4")
            # tail: SWDGE hop is ~4us latency; sync HWDGE is fast there
            (nc.sync if cover else nc.gpsimd).dma_start(d4[:, :],
                                                        d_sb[0:1, :])
            if cover:
                psw = None
                for _ in range(3):
                    psw = psP.tile([P, 256], F32, tag="psP")
                    nc.tensor.matmul(psw[0:D, :], ones1[:, :],
                                     d_sb[0:1, 0:256],
                                     start=True, stop=True)
                nc.vector.tensor_copy(d_sb[0:1, 0:1], psw[0:1, 0:1])
            ps4 = psP.tile([P, GT], F32, tag="psP")
            nc.tensor.transpose(ps4[0:P, 0:GT], d4[:, :], ident4[:, :])
            recip = small.tile([P, GT], F32, tag="recip")
            nc.vector.reciprocal(recip[:, :], ps4[0:P, 0:GT])
            num_sb = small.tile([P, QG], BF16, tag="numsb")
            if cover:
                # tail critical path: produce both halves immediately on
                # the two drain engines in parallel
                nc.vector.tensor_copy(num_sb[0:D, :], psn[0:D, :])
                nc.scalar.copy(num_sb[D:P, :], psn[0:D, :])
            else:
                nc.vector.tensor_copy(num_sb[0:D, :], psn[0:D, :])
                # replica to partitions 64-127 rides the SWDGE queue
                nc.gpsimd.dma_start(num_sb[D:P, :], num_sb[0:D, :])
            return num_sb, recip

        def out_pair(qg, t, num_sb, recip, drain="v", split=False):
            """One q-tile's two E-halves as a concurrent row-tiled pair,
            drained by a single fused tensor_scalar (recip folded in)."""
            i = qg * GT + t  # global q-tile index
            o_t = outp.tile([P, E], BF16, tag="o")
            po = psB.tile([P, 2 * QG], F32, tag="pair", name="po")
            nc.tensor.matmul(po[:, 0:QG],
                             num_sb[0:D, t * P:(t + 1) * P],
                             wv2_sb[0:D, 0:QG],
                             start=True, stop=True)
            nc.tensor.matmul(po[:, QG:2 * QG],
                             num_sb[D:P, t * P:(t + 1) * P],
                             wv2_sb[D:P, QG:2 * QG],
                             start=True, stop=True)
            if split:
                # tail: halve the drain latency across ACT+DVE and ship
                # each half as soon as it lands
                nc.scalar.activation(o_t[:, 0:QG], po[:, 0:QG], COPY_FN,
                                     scale=recip[:, t:t + 1])
                nc.vector.tensor_scalar_mul(o_t[:, QG:E], po[:, QG:2 * QG],
                                            recip[:, t:t + 1])
                hw_engs[t % 2].dma_start(out[i * P:(i + 1) * P, 0:QG],
                                         o_t[:, 0:QG])
                hw_engs[(t + 1) % 2].dma_start(out[i * P:(i + 1) * P, QG:E],
                                               o_t[:, QG:E])
            else:
                if drain == "v":
                    nc.vector.tensor_scalar_mul(o_t[:, :], po[:, :],
                                                recip[:, t:t + 1])
                else:
                    nc.scalar.activation(o_t[:, :], po[:, :], COPY_FN,
                                         scale=recip[:, t:t + 1])
                # ship halves on both queues: halves the tile's ship
                # latency and keeps the small kt/qt_hi/d4 hops from
                # queuing behind a full 256KB transfer
                hw_engs[t % 2].dma_start(out[i * P:(i + 1) * P, 0:QG],
                                         o_t[:, 0:QG])
                hw_engs[(t + 1) % 2].dma_start(out[i * P:(i + 1) * P, QG:E],
                                               o_t[:, QG:E])

        # Software-pipelined schedule: strips for group g+1 are produced one
        # full period ahead, so the numerator matmuls of period g always read
        # exp'd data - TensorE never waits on ACT latency.
        proj_pass_qk(0)
        # defer groups 2-3 (not needed until ~period 1): a WAW byte-dep
        # on qkt (written when proj(0) drains) holds these back so the
        # contended early HBM window carries only groups 0-1.
        nc.vector.tensor_copy(xT[0:1, 2, 0, 0:1], qkt_sb[0:1, 0:1])
        nc.vector.tensor_copy(xT[0:1, 3, 0, 0:1], qkt_sb[0:1, 0:1])
        nc.sync.dma_start(xT[:, 2], xtv[:, 2])
        nc.sync.dma_start(xT[:, 3], xtv[:, 3])
        # consts after the first QK pass: keeps the gpsimd queue clear so
        # the kt(0)/qt_hi(0) re-base DMAs land right behind the weights
        make_identity(nc, ident[:, :])
        make_identity(nc, ident4[:, :])
        make_upper_triangular(nc, tri[:, :], val=1.0, diag=True)
        proj_pass_v1(0)
        vp_transp(0)
        # Trailing numerator for the LAST group: its strip->numerator
        # matmuls are emitted during period 2 (lagging the strip stream so
        # they never wait on exp), leaving only the final few for the
        # drain-limited last period.
        LAG = 6
        tail_num = {"psn": None, "done": 0}

        def num_tail_advance(ents, upto):
            upto = min(upto, len(ents))
            if tail_num["done"] >= upto:
                return
            if tail_num["psn"] is None:
                tail_num["psn"] = psN.tile([D + 1, QG], F32, tag="psn",
                                           name="psn3")
            psn3 = tail_num["psn"]
            for (j, pt_ap, lo) in ents[tail_num["done"]:upto]:
                nc.tensor.matmul(
                    psn3[:, lo:QG], vp_sb[:, j, :], pt_ap,
                    start=(j == 0), stop=(j == NS - 1))
            tail_num["done"] = upto

        entries = []
        for j in range(0, GT, 2):
            entries.extend(emit_strip_pair(0, j))
        nr = {}
        for g in range(NQG - 1):
            n_st = (g + 1) * GT
            items = []
            ng = g + 1

            def out_item(t, idx):
                pn, pr = nr[g - 1]
                # mid-phase drains stay on DVE: an ACT drain sits in the
                # FIFO ahead of the next exp and delays the strip stream
                items.insert(idx, lambda t=t, pn=pn, pr=pr, g2=g - 1:
                             out_pair(g2, t, pn, pr, drain="v"))

            # psP users (proj parts, transposes, keep-alive) stay
            # contiguous in emission order modulo non-psP items, so the
            # two-part accumulating passes are never broken by another
            # psP slot allocation mid-pass.
            items.append(lambda ng=ng: proj_pass_qk(ng, 0))
            items.append(lambda ng=ng: proj_pass_qk(ng, 1))
            items.append(lambda ng=ng: proj_pass_v1(ng, 0))
            items.append(lambda ng=ng: proj_pass_v1(ng, 1))
            items.append(lambda ng=ng: vp_transp(ng, 0))
            items.append(lambda ng=ng: vp_transp(ng, 1))
            if g - 1 >= 0:
                # interleave the out pairs between the pass parts
                out_item(0, 1)
                out_item(1, 3)
                out_item(2, 5)
                out_item(3, 7)
            next_entries = []
            for j in range(0, (g + 2) * GT, 2):
                if ng == NQG - 1:
                    items.append(
                        lambda j=j, g2=ng, acc=next_entries:
                        (acc.extend(emit_strip_pair(g2, j)),
                         num_tail_advance(acc, len(acc) - LAG)))
                else:
                    items.append(
                        lambda j=j, g2=ng, acc=next_entries:
                        acc.extend(emit_strip_pair(g2, j)))
            # trailing keep-alive so the HAM MID window never sees an
            # idle PE while the period's consumers (exp/drains) catch up
            items.append(lambda: emit_warm(2))
            psn = psN.tile([D + 1, QG], F32, tag="psn")
            ii = 0
            for (j, pt_ap, lo) in entries:
                nc.tensor.matmul(
                    psn[:, lo:QG], vp_sb[:, j, :], pt_ap,
                    start=(j == 0), stop=(j == n_st - 1))
                if ii < len(items):
                    items[ii]()
                    ii += 1
            while ii < len(items):
                items[ii]()
                ii += 1
            nr[g] = emit_epilogue(g, psn)
            entries = next_entries
        # final period: drain the remaining trailing numerator MMs with the
        # previous group's out pairs interleaved, then the covered epilogue.
        # Keep-alive dummies between pairs hold the HAM clock gate at 8/8.
        pn, pr = nr[NQG - 2]
        num_tail_advance(entries, NS - 4)
        out_pair(NQG - 2, 0, pn, pr, split=True)
        num_tail_advance(entries, NS - 2)
        out_pair(NQG - 2, 1, pn, pr, split=True)
        num_tail_advance(entries, NS)
        # final epilogue first - its DMA round trip is the tail's critical
        # path; the remaining out(2) pairs execute under it as real cover
        nr[NQG - 1] = emit_epilogue(NQG - 1, tail_num["psn"], cover=True)
        out_pair(NQG - 2, 2, pn, pr, split=True)
        out_pair(NQG - 2, 3, pn, pr, split=True)
        num_sb, recip = nr[NQG - 1]
        for t in range(GT):
            out_pair(NQG - 1, t, num_sb, recip, split=True)

_CACHE = {}


def _get_compiled():
    if "nc" not in _CACHE:
        nc = bacc.Bacc("TRN2", target_bir_lowering=False, debug=False,
                       enable_asserts=False, num_devices=B)
        build_kernel(nc)
        nc.compile()
        _CACHE["nc"] = nc
    return _CACHE["nc"]


def _prep_w(w):
    """[E, M] -> pre-tiled [128, NE*M] bf16 with w'[p, c*M+m] = w[c*128+p, m]."""
    w = np.asarray(w, dtype=np.float32)
    m = w.shape[1]
    return np.ascontiguousarray(
        w.reshape(NE, P, m).transpose(1, 0, 2).reshape(P, NE * m)
        .astype(ml_dtypes.bfloat16))


def _prep_x(x1):
    """[S, E] -> [128, NQG*NE*QG] bf16, x'[p, g*NE*QG + c*QG + s] =
    x[g*QG+s, c*128+p]."""
    return np.ascontiguousarray(
        x1.reshape(NQG, QG, NE, P).transpose(3, 0, 2, 1).reshape(P, -1)
        .astype(ml_dtypes.bfloat16))


def _run(inputs, trace=False, tmpdir=None):
    nc = _get_compiled()
    bf16 = ml_dtypes.bfloat16
    x = np.asarray(inputs["x"], dtype=np.float32)
    wqk = np.concatenate(
        [np.asarray(inputs["w_q"], dtype=np.float32) * SCALE,
         np.asarray(inputs["w_k"], dtype=np.float32)], axis=1)  # [E, 128]
    w = {
        "w_qk": _prep_w(wqk),
        "w_v1": _prep_w(np.asarray(inputs["w_v1"], dtype=np.float32)),
        "w_v2": np.ascontiguousarray(
            np.asarray(inputs["w_v2"], dtype=np.float32).astype(bf16)),
    }
    in_maps = [dict(x_t=_prep_x(x[i]), **w) for i in range(B)]
    res = bass_utils.run_bass_kernel_spmd(
        nc, in_maps, core_ids=list(range(B)), trace=trace, tmpdir=tmpdir,
    )
    outs = np.stack([np.asarray(res.results[i]["out"]) for i in range(B)])
    return outs.astype(np.float32), res


def kernel(**inputs) -> np.ndarray:
    outs, _ = _run(inputs, trace=False)
    return outs


# revision 26
# speedup vs baseline: 1.1031x; 1.0099x over previous
"""Trainium2 Bass kernel: single-head causal attention, data-parallel over batch.

Per core (one batch element):
    Q = x @ w_q; K = x @ w_k; V = (x @ w_v1) @ w_v2
    out = softmax_causal(Q K^T / sqrt(64)) @ V

Sharding: batch 8 -> one element per NeuronCore, weights replicated.

Design notes (v2 of this kernel; hardware-measured 76.6us vs 84.5us v1):
- Low-rank reassociation: attn @ V = (attn @ Vp) @ w_v2 (rank 64), so the
  numerator GEMM contracts to width 64 instead of 1024.
- Scores computed transposed (S^T = K Q^T) so P^T = exp(S^T) lands in the
  lhsT layout of the numerator matmul; a ones column on Vp makes row 64
  of the numerator the softmax denominator for free.
- PE-array ROW TILING for the K=64 matmuls: score strips are emitted as
  concurrent pairs - even strip in array rows 0-63 (kt_sb + Q^T at
  partitions 0-63), odd strip in rows 64-127 (K^T read in place from
  qkt_sb[64:128] + a re-based Q^T copy at partitions 64-127). Verified on
  HW: the pair's matmuls start ~6ns apart when no semaphore wait blocks
  the second. The pair lands in one [128,1024] PSUM tile spanning two
  banks, so a single ACT exp drains both strips (ACT instruction count
  nearly halves - ACT is the mid-phase critical engine).
- Out-GEMM (K=64) row-paired the same way: the two E-halves of a q-tile
  run in rows 0-63 / 64-127 against partition-replicated num / w_v2 into
  one [128,1024] pair, drained by ONE tensor_scalar with recip fused.
- PSUM: one double-buffered 2-bank pair pool shared by score and out
  pairs. bufs=1 self-chained producer->consumer->producer and left the
  PE idle long enough for the HAM clock gate to re-throttle to 1.2GHz
  (the dominant failure mode of every slower variant of this kernel).
- kt/qt_hi re-base copies ride the HWDGE queues (the SWDGE hop measured
  ~4us latency and stalled the first strip pairs of each group).
- Head: groups 2-3 deferred behind a WAW byte-dep on qkt so the early
  HBM window (8 cores contending, per-queue ~130GB/s) carries only
  groups 0-1; 36 N=256 warmup matmuls cover DMA-wait so the clock gate
  is at 8/8 when real work starts. N=64 fillers do NOT register as HAM
  activity - only chunky N>=256 matmuls hold the gate.
- Tail: the final epilogue's d4 round trip rides the sync HWDGE queue
  and is bridged by data-dependent cover matmuls; drains split ACT+DVE.
- Output written bf16 (host upcasts); well inside tolerance.
"""

import os
import sys

import numpy as np

for _p in ("/opt/trn_rl_repo", "/root/.axon_site/_ro/trn_rl_repo"):
    if os.path.isdir(_p) and _p not in sys.path:
        sys.path.insert(0, _p)
os.environ.setdefault("MYCRO_LOCAL_CACHE", "1")

import ml_dtypes  # noqa: E402
import concourse.bass as bass  # noqa: E402
import concourse.mybir as mybir  # noqa: E402
import concourse.tile as tile  # noqa: E402
from concourse import bacc  # noqa: E402
from concourse import bass_utils  # noqa: E402
from concourse.masks import make_identity, make_upper_triangular  # noqa: E402

F32 = mybir.dt.float32
BF16 = mybir.dt.bfloat16

B, S, E, D = 8, 2048, 1024, 64
P = 128
NS = S // P       # 16 s/q tiles
NE = E // P       # 8 E-chunks (projection contraction)
QG = 512          # q-group width
NQG = S // QG     # 4 q-groups
GT = QG // P      # 4 q-tiles per group
SCALE = D ** -0.5
EXP_FN = mybir.ActivationFunctionType.Exp
COPY_FN = mybir.ActivationFunctionType.Copy
NWARM = 36        # dummy matmuls to warm the PE clock gate during loads


def build_kernel(nc):
    # x pre-tiled on host: x_t[p, g*NE*QG + c*QG + s] = x[g*QG+s, c*128+p]
    x_t = nc.dram_tensor("x_t", (P, NQG * NE * QG), BF16,
                         kind="ExternalInput").ap()
    # w_qk pre-tiled: w_qk[p, c*128 + m] = [w_q*scale | w_k][c*128+p, m]
    w_qk = nc.dram_tensor("w_qk", (P, NE * P), BF16, kind="ExternalInput").ap()
    w_v1 = nc.dram_tensor("w_v1", (P, NE * D), BF16, kind="ExternalInput").ap()
    w_v2 = nc.dram_tensor("w_v2", (D, E), BF16, kind="ExternalInput").ap()
    out = nc.dram_tensor("out", (S, E), BF16, kind="ExternalOutput").ap()

    with tile.TileContext(nc) as tc:
        _body(tc, nc, x_t, w_qk, w_v1, w_v2, out)


def _body(tc, nc, x_t, w_qk, w_v1, w_v2, out):
    from contextlib import ExitStack

    with ExitStack() as ctx:
        const = ctx.enter_context(tc.tile_pool(name="const", bufs=1))
        big = ctx.enter_context(tc.tile_pool(name="big", bufs=1))
        # pt pairs live from exp until their last numerator read; sized
        # above peak-live so pool-reuse WARs never stall the producers
        ptp = ctx.enter_context(tc.tile_pool(name="ptp", bufs=14))
        outp = ctx.enter_context(tc.tile_pool(name="outp", bufs=6))
        small = ctx.enter_context(tc.tile_pool(name="small", bufs=8))
        # PSUM budget (8 banks): one double-buffered 2-bank pair pool
        # shared by score pairs AND out pairs (4) + psP 2x1 + psN 2x1.
        # Two slots mean a pair's consumer (exp / drain) never gates the
        # NEXT pair's matmuls - the engines each keep a backlog.
        psB = ctx.enter_context(tc.tile_pool(name="psB", bufs=2, space="PSUM"))
        psP = ctx.enter_context(tc.tile_pool(name="psP", bufs=2, space="PSUM"))
        psN = ctx.enter_context(tc.tile_pool(name="psN", bufs=2, space="PSUM"))

        # ---- warm-up operand: memset immediately, no DMA dependency ----
        wu = const.tile([P, 256], BF16, tag="wu")
        nc.vector.memset(wu[:, :], 0.001)

        # ---- weight + x^T loads ----
        xT = big.tile([P, NQG, NE, QG], BF16, tag="xT")
        xtv = x_t.rearrange("p (g c s) -> p g c s", g=NQG, c=NE)
        wqk_sb = const.tile([P, NE, P], BF16, tag="wqk")
        wv1_sb = const.tile([P, NE, D], BF16, tag="wv1")
        # w_v2 replicated into both partition halves: row-paired out GEMMs
        # read rhs at partitions 0-63 (eh=0) and 64-127 (eh=1)
        wv2_sb = const.tile([P, E], BF16, tag="wv2")
        # wqk split across both HWDGE queues so x group 0 doesn't wait
        # behind the full weight transfer
        wqkv = w_qk.rearrange("p (c m) -> p c m", m=P)
        h = NE // 2
        nc.sync.dma_start(wqk_sb[:, 0:h, :], wqkv[:, 0:h, :])
        nc.scalar.dma_start(wqk_sb[:, h:NE, :], wqkv[:, h:NE, :])
        # x group 0 as one half per HWDGE queue (4KB/partition lines)
        nc.scalar.dma_start(xT[:, 0, 0:h, :], xtv[:, 0, 0:h, :])
        nc.sync.dma_start(xT[:, 0, h:NE, :], xtv[:, 0, h:NE, :])
        hw_engs = (nc.sync, nc.scalar)
        # group 1 next on both queues - needed early (proj(1) in period 0)
        nc.scalar.dma_start(xT[:, 1, 0:h, :], xtv[:, 1, 0:h, :])
        nc.sync.dma_start(xT[:, 1, h:NE, :], xtv[:, 1, h:NE, :])
        nc.gpsimd.dma_start(wv1_sb[:, :, :],
                            w_v1.rearrange("p (c d) -> p c d", d=D))
        nc.gpsimd.dma_start(wv2_sb[0:D, :], w_v2)
        nc.gpsimd.dma_start(wv2_sb[D:P, :], w_v2)

        ident = const.tile([D, D], BF16, tag="ident")
        ident4 = const.tile([GT, GT], F32, tag="ident4")
        tri = const.tile([P, P], BF16, tag="tri")
        ones1 = const.tile([1, D], F32, tag="ones1")
        nc.vector.memset(ones1[:, :], 1.0)
        # tri[s, q] = 1 where s <= q else 0 (valid causal region, S^T layout)

        def emit_warm(n, width=256):
            """Dummy matmuls to keep the PE clock gate at 8/8."""
            psw = None
            for _ in range(n):
                psw = psP.tile([P, 256], F32, tag="psP")
                nc.tensor.matmul(psw[:, 0:width], wu[:, 0:P],
                                 wu[:, 0:width], start=True, stop=True)
            # token reader so the verifier sees the results consumed
            nc.vector.tensor_copy(wu[0:1, 0:1], psw[0:1, 0:1])

        # ---- PE warm-up: dummy matmuls while DMAs stream in ----
        # N=64 fillers were tried and do NOT register as busy in the HAM
        # activity window (50% duty cycle reads as idle); only chunky
        # N=256 back-to-back matmuls hold the clock gate.
        emit_warm(NWARM)

        qkt_sb = big.tile([P, S], BF16, tag="qkt")
        kt_sb = big.tile([D, S], BF16, tag="kt")
        # Q^T re-based to partitions 64-127 (rhs of odd score strips)
        qt_hi = big.tile([P, S], BF16, tag="qthi")
        vpt_sb = big.tile([D, S], BF16, tag="vpt")
        # Vp tile-wise as [s, 64+1] (numerator lhsT); ones column -> denom row
        vp_sb = big.tile([P, NS, D + 1], BF16, tag="vp")
        nc.vector.memset(vp_sb[:, :, D], 1.0)

        proj_ps = {}

        def proj_pass_qk(ng, part=None):
            """part=0/1 emits half the chunks (finer PE interleave
            granularity); part=None emits the whole pass."""
            sl = slice(ng * QG, (ng + 1) * QG)
            if part != 1:
                proj_ps["qk", ng] = psP.tile([P, QG], F32, tag="psP", name="psqk")
            ps = proj_ps["qk", ng]
            lo = 0 if part != 1 else NE // 2
            hi = NE // 2 if part == 0 else NE
            for ec in range(lo, hi):
                nc.tensor.matmul(
                    ps[:, :], wqk_sb[:, ec, :], xT[:, ng, ec, :],
                    start=(ec == 0), stop=(ec == NE - 1))
            if part == 0:
                return
            nc.vector.tensor_copy(qkt_sb[:, sl], ps[:, :])
            # strip pair operands: K^T at partitions 0-63 (even strips),
            # Q^T at partitions 64-127 (odd strips). HWDGE queues: the
            # SWDGE hop measured ~4us latency and stalled the first strips
            nc.sync.dma_start(kt_sb[:, sl], qkt_sb[D:P, sl])
            nc.scalar.dma_start(qt_hi[D:P, sl], qkt_sb[0:D, sl])

        def proj_pass_v1(ng, part=None):
            sl = slice(ng * QG, (ng + 1) * QG)
            if part != 1:
                proj_ps["v1", ng] = psP.tile([P, QG], F32, tag="psP", name="psv1")
            ps = proj_ps["v1", ng]
            lo = 0 if part != 1 else NE // 2
            hi = NE // 2 if part == 0 else NE
            for ec in range(lo, hi):
                nc.tensor.matmul(
                    ps[0:D, :], wv1_sb[:, ec, :], xT[:, ng, ec, :],
                    start=(ec == 0), stop=(ec == NE - 1))
            if part == 0:
                return
            nc.scalar.copy(vpt_sb[:, sl], ps[0:D, :])

        def vp_transp(ng, part=None):
            lo = ng * GT + (2 if part == 1 else 0)
            hi = ng * GT + (2 if part == 0 else GT)
            for st in range(lo, hi):
                pst = psP.tile([P, D], BF16, tag="psP")
                nc.tensor.transpose(pst[0:P, 0:D],
                                    vpt_sb[:, st * P:(st + 1) * P],
                                    ident[:, :])
                nc.vector.tensor_copy(vp_sb[:, st, 0:D], pst[0:P, 0:D])

        def _lo(qg, j):
            dt_blk = j - qg * GT
            return dt_blk * P if 0 < dt_blk < GT else 0

        def emit_strip_pair(qg, j):
            """Strips j (even) and j+1 as a concurrent row-tiled pair in
            one 2-bank PSUM tile, drained by a single fused exp when the
            written region is contiguous."""
            lo0, lo1 = _lo(qg, j), _lo(qg, j + 1)
            ps = psB.tile([P, 2 * QG], F32, tag="pair")
            # even strip -> array rows 0-63
            nc.tensor.matmul(
                ps[:, lo0:QG],
                kt_sb[:, j * P:(j + 1) * P],
                qkt_sb[0:D, qg * QG + lo0:(qg + 1) * QG],
                start=True, stop=True,
            )
            # odd strip -> array rows 64-127 (K^T in place, Q^T replica)
            nc.tensor.matmul(
                ps[:, QG + lo1:2 * QG],
                qkt_sb[D:P, (j + 1) * P:(j + 2) * P],
                qt_hi[D:P, qg * QG + lo1:(qg + 1) * QG],
                start=True, stop=True,
            )
            pt = ptp.tile([P, 2 * QG], BF16, tag="pt")
            if lo0 == 0 and lo1 == 0:
                nc.scalar.activation(pt[:, :], ps[:, :], EXP_FN)
            else:
                nc.scalar.activation(pt[:, lo0:QG], ps[:, lo0:QG], EXP_FN)
                nc.scalar.activation(pt[:, QG + lo1:], ps[:, QG + lo1:],
                                     EXP_FN)
            out = []
            for jj, off, lo in ((j, 0, lo0), (j + 1, QG, lo1)):
                dt_blk = jj - qg * GT
                if 0 <= dt_blk < GT:
                    # mask the diagonal 128x128 block (cols < lo are never
                    # read: numerator MMs are lo-trimmed)
                    nc.gpsimd.tensor_mul(
                        pt[:, off + dt_blk * P:off + (dt_blk + 1) * P],
                        pt[:, off + dt_blk * P:off + (dt_blk + 1) * P],
                        tri[:, :],
                    )
                out.append((jj, pt[:, off + lo:off + QG], lo))
            return out

        def emit_epilogue(qg, psn, cover=False):
            """Denominator row -> per-partition recip; numerator -> bf16
            replicated to both partition halves (row-paired out GEMM).
            cover=True: bridge the d4 DMA round trip with dummy matmuls
            that READ d_sb so the scheduler cannot hoist them."""
            d_sb = small.tile([1, QG], F32, tag="dsb")
            if cover:  # tail: ACT is free; DVE still drains out tiles
                nc.scalar.copy(d_sb[:, :], psn[D:D + 1, :])
            else:
                nc.vector.tensor_copy(d_sb[:, :], psn[D:D + 1, :])
            d4 = small.tile([GT, P], F32, tag="d4")
            # sync HWDGE always: the SWDGE hop's ~4us latency delivered
            # recip a period late and stalled the out pairs behind it
            nc.sync.dma_start(d4[:, :], d_sb[0:1, :])
            if cover:
                psw = None
                for _ in range(5):
                    psw = psP.tile([P, 256], F32, tag="psP")
                    nc.tensor.matmul(psw[0:D, :], ones1[:, :],
                                     d_sb[0:1, 0:256],
                                     start=True, stop=True)
                nc.vector.tensor_copy(d_sb[0:1, 0:1], psw[0:1, 0:1])
            ps4 = psP.tile([P, GT], F32, tag="psP")
            nc.tensor.transpose(ps4[0:P, 0:GT], d4[:, :], ident4[:, :])
            recip = small.tile([P, GT], F32, tag="recip")
            nc.vector.reciprocal(recip[:, :], ps4[0:P, 0:GT])
            num_sb = small.tile([P, QG], BF16, tag="numsb")
            if cover:
                # tail critical path: produce both halves immediately on
                # the two drain engines in parallel
                nc.vector.tensor_copy(num_sb[0:D, :], psn[0:D, :])
                nc.scalar.copy(num_sb[D:P, :], psn[0:D, :])
            else:
                nc.vector.tensor_copy(num_sb[0:D, :], psn[0:D, :])
                # replica to partitions 64-127 rides the SWDGE queue
                nc.gpsimd.dma_start(num_sb[D:P, :], num_sb[0:D, :])
            return num_sb, recip

        def out_pair(qg, t, num_sb, recip, drain="v", split=False):
            """One q-tile's two E-halves as a concurrent row-tiled pair,
            drained by a single fused tensor_scalar (recip folded in)."""
            i = qg * GT + t  # global q-tile index
            o_t = outp.tile([P, E], BF16, tag="o")
            po = psB.tile([P, 2 * QG], F32, tag="pair", name="po")
            nc.tensor.matmul(po[:, 0:QG],
                             num_sb[0:D, t * P:(t + 1) * P],
                             wv2_sb[0:D, 0:QG],
                             start=True, stop=True)
            nc.tensor.matmul(po[:, QG:2 * QG],
                             num_sb[D:P, t * P:(t + 1) * P],
                             wv2_sb[D:P, QG:2 * QG],
                             start=True, stop=True)
            if split:
                # tail: halve the drain latency across ACT+DVE and ship
                # each half as soon as it lands
                nc.scalar.activation(o_t[:, 0:QG], po[:, 0:QG], COPY_FN,
                                     scale=recip[:, t:t + 1])
                nc.vector.tensor_scalar_mul(o_t[:, QG:E], po[:, QG:2 * QG],
                                            recip[:, t:t + 1])
                hw_engs[t % 2].dma_start(out[i * P:(i + 1) * P, 0:QG],
                                         o_t[:, 0:QG])
                hw_engs[(t + 1) % 2].dma_start(out[i * P:(i + 1) * P, QG:E],
                                               o_t[:, QG:E])
            else:
                if drain == "v":
                    nc.vector.tensor_scalar_mul(o_t[:, :], po[:, :],
                                                recip[:, t:t + 1])
                else:
                    nc.scalar.activation(o_t[:, :], po[:, :], COPY_FN,
                                         scale=recip[:, t:t + 1])
                # ship halves on both queues: halves the tile's ship
                # latency and keeps the small kt/qt_hi/d4 hops from
                # queuing behind a full 256KB transfer
                hw_engs[t % 2].dma_start(out[i * P:(i + 1) * P, 0:QG],
                                         o_t[:, 0:QG])
                hw_engs[(t + 1) % 2].dma_start(out[i * P:(i + 1) * P, QG:E],
                                               o_t[:, QG:E])

        # Software-pipelined schedule: strips for group g+1 are produced one
        # full period ahead, so the numerator matmuls of period g always read
        # exp'd data - TensorE never waits on ACT latency.
        proj_pass_qk(0)
        # defer groups 2-3 (not needed until ~period 1): a WAW byte-dep
        # on qkt (written when proj(0) drains) holds these back so the
        # contended early HBM window carries only groups 0-1.
        nc.vector.tensor_copy(xT[0:1, 2, 0, 0:1], qkt_sb[0:1, 0:1])
        nc.vector.tensor_copy(xT[0:1, 3, 0, 0:1], qkt_sb[0:1, 0:1])
        nc.sync.dma_start(xT[:, 2], xtv[:, 2])
        nc.sync.dma_start(xT[:, 3], xtv[:, 3])
        # consts after the first QK pass: keeps the gpsimd queue clear so
        # the kt(0)/qt_hi(0) re-base DMAs land right behind the weights
        make_identity(nc, ident[:, :])
        make_identity(nc, ident4[:, :])
        make_upper_triangular(nc, tri[:, :], val=1.0, diag=True)
        proj_pass_v1(0)
        vp_transp(0)
        # Trailing numerator for the LAST group: its strip->numerator
        # matmuls are emitted during period 2 (lagging the strip stream so
        # they never wait on exp), leaving only the final few for the
        # drain-limited last period.
        LAG = 6
        tail_num = {"psn": None, "done": 0}

        def num_tail_advance(ents, upto):
            upto = min(upto, len(ents))
            if tail_num["done"] >= upto:
                return
            if tail_num["psn"] is None:
                tail_num["psn"] = psN.tile([D + 1, QG], F32, tag="psn",
                                           name="psn3")
            psn3 = tail_num["psn"]
            for (j, pt_ap, lo) in ents[tail_num["done"]:upto]:
                nc.tensor.matmul(
                    psn3[:, lo:QG], vp_sb[:, j, :], pt_ap,
                    start=(j == 0), stop=(j == NS - 1))
            tail_num["done"] = upto

        entries = []
        for j in range(0, GT, 2):
            entries.extend(emit_strip_pair(0, j))
        nr = {}
        for g in range(NQG - 1):
            n_st = (g + 1) * GT
            items = []
            ng = g + 1

            def out_item(t, idx):
                pn, pr = nr[g - 1]
                # mid-phase drains stay on DVE: an ACT drain sits in the
                # FIFO ahead of the next exp and delays the strip stream
                items.insert(idx, lambda t=t, pn=pn, pr=pr, g2=g - 1:
                             out_pair(g2, t, pn, pr, drain="v"))

            # psP users (proj parts, transposes, keep-alive) stay
            # contiguous in emission order modulo non-psP items, so the
            # two-part accumulating passes are never broken by another
            # psP slot allocation mid-pass.
            items.append(lambda ng=ng: proj_pass_qk(ng, 0))
            items.append(lambda ng=ng: proj_pass_qk(ng, 1))
            items.append(lambda ng=ng: proj_pass_v1(ng, 0))
            items.append(lambda ng=ng: proj_pass_v1(ng, 1))
            items.append(lambda ng=ng: vp_transp(ng, 0))
            items.append(lambda ng=ng: vp_transp(ng, 1))
            if g - 1 >= 0:
                # interleave the out pairs between the pass parts
                out_item(0, 1)
                out_item(1, 3)
                out_item(2, 5)
                out_item(3, 7)
            next_entries = []
            for j in range(0, (g + 2) * GT, 2):
                if ng == NQG - 1:
                    items.append(
                        lambda j=j, g2=ng, acc=next_entries:
                        (acc.extend(emit_strip_pair(g2, j)),
                         num_tail_advance(acc, len(acc) - LAG)))
                else:
                    items.append(
                        lambda j=j, g2=ng, acc=next_entries:
                        acc.extend(emit_strip_pair(g2, j)))
            # trailing keep-alive so the HAM MID window never sees an
            # idle PE while the period's consumers (exp/drains) catch up
            items.append(lambda: emit_warm(2))
            psn = psN.tile([D + 1, QG], F32, tag="psn")
            ii = 0
            for (j, pt_ap, lo) in entries:
                nc.tensor.matmul(
                    psn[:, lo:QG], vp_sb[:, j, :], pt_ap,
                    start=(j == 0), stop=(j == n_st - 1))
                if ii < len(items):
                    items[ii]()
                    ii += 1
            while ii < len(items):
                items[ii]()
                ii += 1
            nr[g] = emit_epilogue(g, psn)
            entries = next_entries
        # final period: drain the remaining trailing numerator MMs with the
        # previous group's out pairs interleaved, then the covered epilogue.
        # Keep-alive dummies between pairs hold the HAM clock gate at 8/8.
        pn, pr = nr[NQG - 2]
        num_tail_advance(entries, NS - 4)
        out_pair(NQG - 2, 0, pn, pr, split=True)
        num_tail_advance(entries, NS - 2)
        out_pair(NQG - 2, 1, pn, pr, split=True)
        num_tail_advance(entries, NS)
        # final epilogue first - its DMA round trip is the tail's critical
        # path; the remaining out(2) pairs execute under it as real cover
        nr[NQG - 1] = emit_epilogue(NQG - 1, tail_num["psn"], cover=True)
        out_pair(NQG - 2, 2, pn, pr, split=True)
        out_pair(NQG - 2, 3, pn, pr, split=True)
        num_sb, recip = nr[NQG - 1]
        for t in range(GT):
            out_pair(NQG - 1, t, num_sb, recip, split=True)

_CACHE = {}


def _get_compiled():
    if "nc" not in _CACHE:
        nc = bacc.Bacc("TRN2", target_bir_lowering=False, debug=False,
                       enable_asserts=False, num_devices=B)
        build_kernel(nc)
        nc.compile()
        _CACHE["nc"] = nc
    return _CACHE["nc"]


def _prep_w(w):
    """[E, M] -> pre-tiled [128, NE*M] bf16 with w'[p, c*M+m] = w[c*128+p, m]."""
    w = np.asarray(w, dtype=np.float32)
    m = w.shape[1]
    return np.ascontiguousarray(
        w.reshape(NE, P, m).transpose(1, 0, 2).reshape(P, NE * m)
        .astype(ml_dtypes.bfloat16))


def _prep_x(x1):
    """[S, E] -> [128, NQG*NE*QG] bf16, x'[p, g*NE*QG + c*QG + s] =
    x[g*QG+s, c*128+p]."""
    return np.ascontiguousarray(
        x1.reshape(NQG, QG, NE, P).transpose(3, 0, 2, 1).reshape(P, -1)
        .astype(ml_dtypes.bfloat16))


def _run(inputs, trace=False, tmpdir=None):
    nc = _get_compiled()
    bf16 = ml_dtypes.bfloat16
    x = np.asarray(inputs["x"], dtype=np.float32)
    wqk = np.concatenate(
        [np.asarray(inputs["w_q"], dtype=np.float32) * SCALE,
         np.asarray(inputs["w_k"], dtype=np.float32)], axis=1)  # [E, 128]
    w = {
        "w_qk": _prep_w(wqk),
        "w_v1": _prep_w(np.asarray(inputs["w_v1"], dtype=np.float32)),
        "w_v2": np.ascontiguousarray(
            np.asarray(inputs["w_v2"], dtype=np.float32).astype(bf16)),
    }
    in_maps = [dict(x_t=_prep_x(x[i]), **w) for i in range(B)]
    res = bass_utils.run_bass_kernel_spmd(
        nc, in_maps, core_ids=list(range(B)), trace=trace, tmpdir=tmpdir,
    )
    outs = np.stack([np.asarray(res.results[i]["out"]) for i in range(B)])
    return outs.astype(np.float32), res


def kernel(**inputs) -> np.ndarray:
    outs, _ = _run(inputs, trace=False)
    return outs


# revision 28
# speedup vs baseline: 1.1087x; 1.0050x over previous
"""Trainium2 Bass kernel: single-head causal attention, data-parallel over batch.

Per core (one batch element):
    Q = x @ w_q; K = x @ w_k; V = (x @ w_v1) @ w_v2
    out = softmax_causal(Q K^T / sqrt(64)) @ V

Sharding: batch 8 -> one element per NeuronCore, weights replicated.

Design notes (v2 of this kernel; hardware-measured 76.6us vs 84.5us v1):
- Low-rank reassociation: attn @ V = (attn @ Vp) @ w_v2 (rank 64), so the
  numerator GEMM contracts to width 64 instead of 1024.
- Scores computed transposed (S^T = K Q^T) so P^T = exp(S^T) lands in the
  lhsT layout of the numerator matmul; a ones column on Vp makes row 64
  of the numerator the softmax denominator for free.
- PE-array ROW TILING for the K=64 matmuls: score strips are emitted as
  concurrent pairs - even strip in array rows 0-63 (kt_sb + Q^T at
  partitions 0-63), odd strip in rows 64-127 (K^T read in place from
  qkt_sb[64:128] + a re-based Q^T copy at partitions 64-127). Verified on
  HW: the pair's matmuls start ~6ns apart when no semaphore wait blocks
  the second. The pair lands in one [128,1024] PSUM tile spanning two
  banks, so a single ACT exp drains both strips (ACT instruction count
  nearly halves - ACT is the mid-phase critical engine).
- Out-GEMM (K=64) row-paired the same way: the two E-halves of a q-tile
  run in rows 0-63 / 64-127 against partition-replicated num / w_v2 into
  one [128,1024] pair, drained by ONE tensor_scalar with recip fused.
- PSUM: one double-buffered 2-bank pair pool shared by score and out
  pairs. bufs=1 self-chained producer->consumer->producer and left the
  PE idle long enough for the HAM clock gate to re-throttle to 1.2GHz
  (the dominant failure mode of every slower variant of this kernel).
- kt/qt_hi re-base copies ride the HWDGE queues (the SWDGE hop measured
  ~4us latency and stalled the first strip pairs of each group).
- Head: groups 2-3 deferred behind a WAW byte-dep on qkt so the early
  HBM window (8 cores contending, per-queue ~130GB/s) carries only
  groups 0-1; 36 N=256 warmup matmuls cover DMA-wait so the clock gate
  is at 8/8 when real work starts. N=64 fillers do NOT register as HAM
  activity - only chunky N>=256 matmuls hold the gate.
- Tail: the final epilogue's d4 round trip rides the sync HWDGE queue
  and is bridged by data-dependent cover matmuls; drains split ACT+DVE.
- Output written bf16 (host upcasts); well inside tolerance.
"""

import os
import sys

import numpy as np

for _p in ("/opt/trn_rl_repo", "/root/.axon_site/_ro/trn_rl_repo"):
    if os.path.isdir(_p) and _p not in sys.path:
        sys.path.insert(0, _p)
os.environ.setdefault("MYCRO_LOCAL_CACHE", "1")

import ml_dtypes  # noqa: E402
import concourse.bass as bass  # noqa: E402
import concourse.mybir as mybir  # noqa: E402
import concourse.tile as tile  # noqa: E402
from concourse import bacc  # noqa: E402
from concourse import bass_utils  # noqa: E402
from concourse.masks import make_identity, make_upper_triangular  # noqa: E402

F32 = mybir.dt.float32
BF16 = mybir.dt.bfloat16

B, S, E, D = 8, 2048, 1024, 64
P = 128
NS = S // P       # 16 s/q tiles
NE = E // P       # 8 E-chunks (projection contraction)
QG = 512          # q-group width
NQG = S // QG     # 4 q-groups
GT = QG // P      # 4 q-tiles per group
SCALE = D ** -0.5
EXP_FN = mybir.ActivationFunctionType.Exp
COPY_FN = mybir.ActivationFunctionType.Copy
NWARM = 36        # dummy matmuls to warm the PE clock gate during loads


def build_kernel(nc):
    # x pre-tiled on host: x_t[p, g*NE*QG + c*QG + s] = x[g*QG+s, c*128+p]
    x_t = nc.dram_tensor("x_t", (P, NQG * NE * QG), BF16,
                         kind="ExternalInput").ap()
    # w_qk pre-tiled: w_qk[p, c*128 + m] = [w_q*scale | w_k][c*128+p, m]
    w_qk = nc.dram_tensor("w_qk", (P, NE * P), BF16, kind="ExternalInput").ap()
    w_v1 = nc.dram_tensor("w_v1", (P, NE * D), BF16, kind="ExternalInput").ap()
    w_v2 = nc.dram_tensor("w_v2", (D, E), BF16, kind="ExternalInput").ap()
    out = nc.dram_tensor("out", (S, E), BF16, kind="ExternalOutput").ap()

    with tile.TileContext(nc) as tc:
        _body(tc, nc, x_t, w_qk, w_v1, w_v2, out)


def _body(tc, nc, x_t, w_qk, w_v1, w_v2, out):
    from contextlib import ExitStack

    with ExitStack() as ctx:
        const = ctx.enter_context(tc.tile_pool(name="const", bufs=1))
        big = ctx.enter_context(tc.tile_pool(name="big", bufs=1))
        # pt pairs live from exp until their last numerator read; sized
        # above peak-live so pool-reuse WARs never stall the producers
        ptp = ctx.enter_context(tc.tile_pool(name="ptp", bufs=14))
        outp = ctx.enter_context(tc.tile_pool(name="outp", bufs=8))
        small = ctx.enter_context(tc.tile_pool(name="small", bufs=8))
        # PSUM budget (8 banks): one double-buffered 2-bank pair pool
        # shared by score pairs AND out pairs (4) + psP 2x1 + psN 2x1.
        # Two slots mean a pair's consumer (exp / drain) never gates the
        # NEXT pair's matmuls - the engines each keep a backlog.
        psB = ctx.enter_context(tc.tile_pool(name="psB", bufs=2, space="PSUM"))
        psP = ctx.enter_context(tc.tile_pool(name="psP", bufs=2, space="PSUM"))
        psN = ctx.enter_context(tc.tile_pool(name="psN", bufs=2, space="PSUM"))

        # ---- warm-up operand: memset immediately, no DMA dependency ----
        wu = const.tile([P, 256], BF16, tag="wu")
        nc.vector.memset(wu[:, :], 0.001)

        # ---- weight + x^T loads ----
        xT = big.tile([P, NQG, NE, QG], BF16, tag="xT")
        xtv = x_t.rearrange("p (g c s) -> p g c s", g=NQG, c=NE)
        wqk_sb = const.tile([P, NE, P], BF16, tag="wqk")
        wv1_sb = const.tile([P, NE, D], BF16, tag="wv1")
        # w_v2 replicated into both partition halves: row-paired out GEMMs
        # read rhs at partitions 0-63 (eh=0) and 64-127 (eh=1)
        wv2_sb = const.tile([P, E], BF16, tag="wv2")
        # wqk split across both HWDGE queues so x group 0 doesn't wait
        # behind the full weight transfer
        wqkv = w_qk.rearrange("p (c m) -> p c m", m=P)
        h = NE // 2
        # wqk rides the gpsimd SWDGE queue: the two HWDGE queues only
        # sustain ~90GB/s/core each, and wqk's 128KB ahead of x group 0
        # delayed the first projection by ~1.4us. The weights tolerate
        # SWDGE latency (gate only the matmuls, chunk by chunk).
        nc.gpsimd.dma_start(wqk_sb[:, :, :], wqkv[:, :, :])
        # x group 0 as one half per HWDGE queue (4KB/partition lines)
        nc.scalar.dma_start(xT[:, 0, 0:h, :], xtv[:, 0, 0:h, :])
        nc.sync.dma_start(xT[:, 0, h:NE, :], xtv[:, 0, h:NE, :])
        hw_engs = (nc.sync, nc.scalar)
        # group 1 next on both queues - needed early (proj(1) in period 0)
        nc.scalar.dma_start(xT[:, 1, 0:h, :], xtv[:, 1, 0:h, :])
        nc.sync.dma_start(xT[:, 1, h:NE, :], xtv[:, 1, h:NE, :])
        nc.gpsimd.dma_start(wv1_sb[:, :, :],
                            w_v1.rearrange("p (c d) -> p c d", d=D))
        nc.gpsimd.dma_start(wv2_sb[0:D, :], w_v2)
        nc.gpsimd.dma_start(wv2_sb[D:P, :], w_v2)

        ident = const.tile([D, D], BF16, tag="ident")
        ident4 = const.tile([GT, GT], F32, tag="ident4")
        tri = const.tile([P, P], BF16, tag="tri")
        ones1 = const.tile([1, D], F32, tag="ones1")
        nc.vector.memset(ones1[:, :], 1.0)
        # tri[s, q] = 1 where s <= q else 0 (valid causal region, S^T layout)

        def emit_warm(n, width=256):
            """Dummy matmuls to keep the PE clock gate at 8/8."""
            psw = None
            for _ in range(n):
                psw = psP.tile([P, 256], F32, tag="psP")
                nc.tensor.matmul(psw[:, 0:width], wu[:, 0:P],
                                 wu[:, 0:width], start=True, stop=True)
            # token reader so the verifier sees the results consumed
            nc.vector.tensor_copy(wu[0:1, 0:1], psw[0:1, 0:1])

        # ---- PE warm-up: dummy matmuls while DMAs stream in ----
        # N=64 fillers were tried and do NOT register as busy in the HAM
        # activity window (50% duty cycle reads as idle); only chunky
        # N=256 back-to-back matmuls hold the clock gate.
        emit_warm(NWARM)

        qkt_sb = big.tile([P, S], BF16, tag="qkt")
        kt_sb = big.tile([D, S], BF16, tag="kt")
        # Q^T re-based to partitions 64-127 (rhs of odd score strips)
        qt_hi = big.tile([P, S], BF16, tag="qthi")
        vpt_sb = big.tile([D, S], BF16, tag="vpt")
        # Vp tile-wise as [s, 64+1] (numerator lhsT); ones column -> denom row
        vp_sb = big.tile([P, NS, D + 1], BF16, tag="vp")
        nc.vector.memset(vp_sb[:, :, D], 1.0)

        proj_ps = {}

        def proj_pass_qk(ng, part=None):
            """part=0/1 emits half the chunks (finer PE interleave
            granularity); part=None emits the whole pass."""
            sl = slice(ng * QG, (ng + 1) * QG)
            if part != 1:
                proj_ps["qk", ng] = psP.tile([P, QG], F32, tag="psP", name="psqk")
            ps = proj_ps["qk", ng]
            lo = 0 if part != 1 else NE // 2
            hi = NE // 2 if part == 0 else NE
            for ec in range(lo, hi):
                nc.tensor.matmul(
                    ps[:, :], wqk_sb[:, ec, :], xT[:, ng, ec, :],
                    start=(ec == 0), stop=(ec == NE - 1))
            if part == 0:
                return
            nc.vector.tensor_copy(qkt_sb[:, sl], ps[:, :])
            # strip pair operands: K^T at partitions 0-63 (even strips),
            # Q^T at partitions 64-127 (odd strips). HWDGE queues: the
            # SWDGE hop measured ~4us latency and stalled the first strips
            nc.sync.dma_start(kt_sb[:, sl], qkt_sb[D:P, sl])
            nc.scalar.dma_start(qt_hi[D:P, sl], qkt_sb[0:D, sl])

        def proj_pass_v1(ng, part=None):
            sl = slice(ng * QG, (ng + 1) * QG)
            if part != 1:
                proj_ps["v1", ng] = psP.tile([P, QG], F32, tag="psP", name="psv1")
            ps = proj_ps["v1", ng]
            lo = 0 if part != 1 else NE // 2
            hi = NE // 2 if part == 0 else NE
            for ec in range(lo, hi):
                nc.tensor.matmul(
                    ps[0:D, :], wv1_sb[:, ec, :], xT[:, ng, ec, :],
                    start=(ec == 0), stop=(ec == NE - 1))
            if part == 0:
                return
            nc.scalar.copy(vpt_sb[:, sl], ps[0:D, :])

        def vp_transp(ng, part=None):
            lo = ng * GT + (2 if part == 1 else 0)
            hi = ng * GT + (2 if part == 0 else GT)
            for st in range(lo, hi):
                pst = psP.tile([P, D], BF16, tag="psP")
                nc.tensor.transpose(pst[0:P, 0:D],
                                    vpt_sb[:, st * P:(st + 1) * P],
                                    ident[:, :])
                nc.vector.tensor_copy(vp_sb[:, st, 0:D], pst[0:P, 0:D])

        def _lo(qg, j):
            dt_blk = j - qg * GT
            return dt_blk * P if 0 < dt_blk < GT else 0

        def emit_strip_pair(qg, j):
            """Strips j (even) and j+1 as a concurrent row-tiled pair in
            one 2-bank PSUM tile, drained by a single fused exp when the
            written region is contiguous."""
            lo0, lo1 = _lo(qg, j), _lo(qg, j + 1)
            ps = psB.tile([P, 2 * QG], F32, tag="pair")
            # even strip -> array rows 0-63
            nc.tensor.matmul(
                ps[:, lo0:QG],
                kt_sb[:, j * P:(j + 1) * P],
                qkt_sb[0:D, qg * QG + lo0:(qg + 1) * QG],
                start=True, stop=True,
            )
            # odd strip -> array rows 64-127 (K^T in place, Q^T replica)
            nc.tensor.matmul(
                ps[:, QG + lo1:2 * QG],
                qkt_sb[D:P, (j + 1) * P:(j + 2) * P],
                qt_hi[D:P, qg * QG + lo1:(qg + 1) * QG],
                start=True, stop=True,
            )
            pt = ptp.tile([P, 2 * QG], BF16, tag="pt")
            if lo0 == 0 and lo1 == 0:
                nc.scalar.activation(pt[:, :], ps[:, :], EXP_FN)
            else:
                nc.scalar.activation(pt[:, lo0:QG], ps[:, lo0:QG], EXP_FN)
                nc.scalar.activation(pt[:, QG + lo1:], ps[:, QG + lo1:],
                                     EXP_FN)
            out = []
            for jj, off, lo in ((j, 0, lo0), (j + 1, QG, lo1)):
                dt_blk = jj - qg * GT
                if 0 <= dt_blk < GT:
                    # mask the diagonal 128x128 block (cols < lo are never
                    # read: numerator MMs are lo-trimmed)
                    nc.gpsimd.tensor_mul(
                        pt[:, off + dt_blk * P:off + (dt_blk + 1) * P],
                        pt[:, off + dt_blk * P:off + (dt_blk + 1) * P],
                        tri[:, :],
                    )
                out.append((jj, pt[:, off + lo:off + QG], lo))
            return out

        def emit_epilogue(qg, psn, cover=False):
            """Denominator row -> per-partition recip; numerator -> bf16
            replicated to both partition halves (row-paired out GEMM).
            cover=True: bridge the d4 DMA round trip with dummy matmuls
            that READ d_sb so the scheduler cannot hoist them."""
            d_sb = small.tile([1, QG], F32, tag="dsb")
            if cover:  # tail: ACT is free; DVE still drains out tiles
                nc.scalar.copy(d_sb[:, :], psn[D:D + 1, :])
            else:
                nc.vector.tensor_copy(d_sb[:, :], psn[D:D + 1, :])
            d4 = small.tile([GT, P], F32, tag="d4")
            # sync HWDGE always: the SWDGE hop's ~4us latency delivered
            # recip a period late and stalled the out pairs behind it
            nc.sync.dma_start(d4[:, :], d_sb[0:1, :])
            if cover:
                psw = None
                for _ in range(5):
                    psw = psP.tile([P, 256], F32, tag="psP")
                    nc.tensor.matmul(psw[0:D, :], ones1[:, :],
                                     d_sb[0:1, 0:256],
                                     start=True, stop=True)
                nc.vector.tensor_copy(d_sb[0:1, 0:1], psw[0:1, 0:1])
            ps4 = psP.tile([P, GT], F32, tag="psP")
            nc.tensor.transpose(ps4[0:P, 0:GT], d4[:, :], ident4[:, :])
            recip = small.tile([P, GT], F32, tag="recip")
            nc.vector.reciprocal(recip[:, :], ps4[0:P, 0:GT])
            num_sb = small.tile([P, QG], BF16, tag="numsb")
            if cover:
                # tail critical path: produce both halves immediately on
                # the two drain engines in parallel
                nc.vector.tensor_copy(num_sb[0:D, :], psn[0:D, :])
                nc.scalar.copy(num_sb[D:P, :], psn[0:D, :])
            else:
                nc.vector.tensor_copy(num_sb[0:D, :], psn[0:D, :])
                # replica to partitions 64-127 rides the SWDGE queue
                nc.gpsimd.dma_start(num_sb[D:P, :], num_sb[0:D, :])
            return num_sb, recip

        def out_pair(qg, t, num_sb, recip, drain="v", split=False):
            """One q-tile's two E-halves as a concurrent row-tiled pair,
            drained by a single fused tensor_scalar (recip folded in)."""
            i = qg * GT + t  # global q-tile index
            o_t = outp.tile([P, E], BF16, tag="o")
            po = psB.tile([P, 2 * QG], F32, tag="pair", name="po")
            nc.tensor.matmul(po[:, 0:QG],
                             num_sb[0:D, t * P:(t + 1) * P],
                             wv2_sb[0:D, 0:QG],
                             start=True, stop=True)
            nc.tensor.matmul(po[:, QG:2 * QG],
                             num_sb[D:P, t * P:(t + 1) * P],
                             wv2_sb[D:P, QG:2 * QG],
                             start=True, stop=True)
            if split:
                # tail: halve the drain latency across ACT+DVE and ship
                # each half as soon as it lands
                nc.scalar.activation(o_t[:, 0:QG], po[:, 0:QG], COPY_FN,
                                     scale=recip[:, t:t + 1])
                nc.vector.tensor_scalar_mul(o_t[:, QG:E], po[:, QG:2 * QG],
                                            recip[:, t:t + 1])
                hw_engs[t % 2].dma_start(out[i * P:(i + 1) * P, 0:QG],
                                         o_t[:, 0:QG])
                hw_engs[(t + 1) % 2].dma_start(out[i * P:(i + 1) * P, QG:E],
                                               o_t[:, QG:E])
            else:
                if drain == "v":
                    nc.vector.tensor_scalar_mul(o_t[:, :], po[:, :],
                                                recip[:, t:t + 1])
                else:
                    nc.scalar.activation(o_t[:, :], po[:, :], COPY_FN,
                                         scale=recip[:, t:t + 1])
                # ship halves on both queues: halves the tile's ship
                # latency and keeps the small kt/qt_hi/d4 hops from
                # queuing behind a full 256KB transfer
                hw_engs[t % 2].dma_start(out[i * P:(i + 1) * P, 0:QG],
                                         o_t[:, 0:QG])
                hw_engs[(t + 1) % 2].dma_start(out[i * P:(i + 1) * P, QG:E],
                                               o_t[:, QG:E])

        # Software-pipelined schedule: strips for group g+1 are produced one
        # full period ahead, so the numerator matmuls of period g always read
        # exp'd data - TensorE never waits on ACT latency.
        proj_pass_qk(0)
        # defer groups 2-3 (not needed until ~period 1): a WAW byte-dep
        # on qkt (written when proj(0) drains) holds these back so the
        # contended early HBM window carries only groups 0-1.
        nc.vector.tensor_copy(xT[0:1, 2, 0, 0:1], qkt_sb[0:1, 0:1])
        nc.vector.tensor_copy(xT[0:1, 3, 0, 0:1], qkt_sb[0:1, 0:1])
        nc.sync.dma_start(xT[:, 2], xtv[:, 2])
        nc.sync.dma_start(xT[:, 3], xtv[:, 3])
        # consts after the first QK pass: keeps the gpsimd queue clear so
        # the kt(0)/qt_hi(0) re-base DMAs land right behind the weights
        make_identity(nc, ident[:, :])
        make_identity(nc, ident4[:, :])
        make_upper_triangular(nc, tri[:, :], val=1.0, diag=True)
        proj_pass_v1(0)
        vp_transp(0)
        # Trailing numerator for the LAST group: its strip->numerator
        # matmuls are emitted during period 2 (lagging the strip stream so
        # they never wait on exp), leaving only the final few for the
        # drain-limited last period.
        LAG = 4
        tail_num = {"psn": None, "done": 0}

        def num_tail_advance(ents, upto):
            upto = min(upto, len(ents))
            if tail_num["done"] >= upto:
                return
            if tail_num["psn"] is None:
                tail_num["psn"] = psN.tile([D + 1, QG], F32, tag="psn",
                                           name="psn3")
            psn3 = tail_num["psn"]
            for (j, pt_ap, lo) in ents[tail_num["done"]:upto]:
                nc.tensor.matmul(
                    psn3[:, lo:QG], vp_sb[:, j, :], pt_ap,
                    start=(j == 0), stop=(j == NS - 1))
            tail_num["done"] = upto

        entries = []
        for j in range(0, GT, 2):
            entries.extend(emit_strip_pair(0, j))
        nr = {}
        for g in range(NQG - 1):
            n_st = (g + 1) * GT
            items = []
            ng = g + 1

            def out_item(t, idx):
                pn, pr = nr[g - 1]
                # mid-phase drains stay on DVE: an ACT drain sits in the
                # FIFO ahead of the next exp and delays the strip stream
                items.insert(idx, lambda t=t, pn=pn, pr=pr, g2=g - 1:
                             out_pair(g2, t, pn, pr, drain="v"))

            # psP users (proj parts, transposes, keep-alive) stay
            # contiguous in emission order modulo non-psP items, so the
            # two-part accumulating passes are never broken by another
            # psP slot allocation mid-pass.
            items.append(lambda ng=ng: proj_pass_qk(ng, 0))
            items.append(lambda ng=ng: proj_pass_qk(ng, 1))
            items.append(lambda ng=ng: proj_pass_v1(ng, 0))
            items.append(lambda ng=ng: proj_pass_v1(ng, 1))
            items.append(lambda ng=ng: vp_transp(ng, 0))
            items.append(lambda ng=ng: vp_transp(ng, 1))
            if g - 1 >= 0:
                # interleave the out pairs between the pass parts
                out_item(0, 1)
                out_item(1, 3)
                out_item(2, 5)
                out_item(3, 7)
            next_entries = []
            for j in range(0, (g + 2) * GT, 2):
                if ng == NQG - 1:
                    items.append(
                        lambda j=j, g2=ng, acc=next_entries:
                        (acc.extend(emit_strip_pair(g2, j)),
                         num_tail_advance(acc, len(acc) - LAG)))
                else:
                    items.append(
                        lambda j=j, g2=ng, acc=next_entries:
                        acc.extend(emit_strip_pair(g2, j)))
            # trailing keep-alive so the HAM MID window never sees an
            # idle PE while the period's consumers (exp/drains) catch up
            items.append(lambda: emit_warm(2))
            psn = psN.tile([D + 1, QG], F32, tag="psn")
            ii = 0
            for (j, pt_ap, lo) in entries:
                nc.tensor.matmul(
                    psn[:, lo:QG], vp_sb[:, j, :], pt_ap,
                    start=(j == 0), stop=(j == n_st - 1))
                if ii < len(items):
                    items[ii]()
                    ii += 1
            while ii < len(items):
                items[ii]()
                ii += 1
            nr[g] = emit_epilogue(g, psn)
            entries = next_entries
        # final period: drain the remaining trailing numerator MMs with the
        # previous group's out pairs interleaved, then the covered epilogue.
        # Keep-alive dummies between pairs hold the HAM clock gate at 8/8.
        pn, pr = nr[NQG - 2]
        num_tail_advance(entries, NS - 4)
        out_pair(NQG - 2, 0, pn, pr, split=True)
        num_tail_advance(entries, NS - 2)
        out_pair(NQG - 2, 1, pn, pr, split=True)
        num_tail_advance(entries, NS)
        # final epilogue first - its DMA round trip is the tail's critical
        # path; the remaining out(2) pairs execute under it as real cover
        nr[NQG - 1] = emit_epilogue(NQG - 1, tail_num["psn"], cover=True)
        out_pair(NQG - 2, 2, pn, pr, split=True)
        out_pair(NQG - 2, 3, pn, pr, split=True)
        num_sb, recip = nr[NQG - 1]
        for t in range(GT):
            out_pair(NQG - 1, t, num_sb, recip, split=True)

_CACHE = {}


def _get_compiled():
    if "nc" not in _CACHE:
        nc = bacc.Bacc("TRN2", target_bir_lowering=False, debug=False,
                       enable_asserts=False, num_devices=B)
        build_kernel(nc)
        nc.compile()
        _CACHE["nc"] = nc
    return _CACHE["nc"]


def _prep_w(w):
    """[E, M] -> pre-tiled [128, NE*M] bf16 with w'[p, c*M+m] = w[c*128+p, m]."""
    w = np.asarray(w, dtype=np.float32)
    m = w.shape[1]
    return np.ascontiguousarray(
        w.reshape(NE, P, m).transpose(1, 0, 2).reshape(P, NE * m)
        .astype(ml_dtypes.bfloat16))


def _prep_x(x1):
    """[S, E] -> [128, NQG*NE*QG] bf16, x'[p, g*NE*QG + c*QG + s] =
    x[g*QG+s, c*128+p]."""
    return np.ascontiguousarray(
        x1.reshape(NQG, QG, NE, P).transpose(3, 0, 2, 1).reshape(P, -1)
        .astype(ml_dtypes.bfloat16))


def _run(inputs, trace=False, tmpdir=None):
    nc = _get_compiled()
    bf16 = ml_dtypes.bfloat16
    x = np.asarray(inputs["x"], dtype=np.float32)
    wqk = np.concatenate(
        [np.asarray(inputs["w_q"], dtype=np.float32) * SCALE,
         np.asarray(inputs["w_k"], dtype=np.float32)], axis=1)  # [E, 128]
    w = {
        "w_qk": _prep_w(wqk),
        "w_v1": _prep_w(np.asarray(inputs["w_v1"], dtype=np.float32)),
        "w_v2": np.ascontiguousarray(
            np.asarray(inputs["w_v2"], dtype=np.float32).astype(bf16)),
    }
    in_maps = [dict(x_t=_prep_x(x[i]), **w) for i in range(B)]
    res = bass_utils.run_bass_kernel_spmd(
        nc, in_maps, core_ids=list(range(B)), trace=trace, tmpdir=tmpdir,
    )
    outs = np.stack([np.asarray(res.results[i]["out"]) for i in range(B)])
    return outs.astype(np.float32), res


def kernel(**inputs) -> np.ndarray:
    outs, _ = _run(inputs, trace=False)
    return outs
